# revision 26
# baseline (speedup 1.0000x reference)
"""Trainium2 Bass kernel for nn_IrradiationSingleTimestep.

Phase-field irradiation single timestep: 3 fields (cv, ci, eta) of shape
[8, 1024, 1024], 5-point periodic Laplacians (two levels), pointwise
thermodynamics with logs, clipped Euler update.

Sharding: batch-parallel, one batch image per NeuronCore (8 cores).

Strategy (bf16 compute, engine-balanced):
- Host precomputes max(cv,eps), max(ci,eps), max(1-cv-ci,eps) in fp32 and
  rounds to bf16.  The 1-cv-ci cancellation is the only absolute-error
  amplifier through ln(); doing it on host keeps ln() errors relative
  (~2^-9) everywhere.  Logs run on the Act engine with fp32 outputs;
  the pass-2 update chain is fp32 for output-error margin.
- DVE perf modes: tensor_scalar bf16 packed = 4x, tensor_tensor bf16
  packed = 2x, fp32 all-SBUF tensor_scalar = 2x, scalar_tensor_tensor =
  no fast mode (and ILLEGAL on GpSimd in the real ISA).  So tensor x
  tensor ops are plain bf16 tensor_tensors on DVE, standalone scalings
  and clips are tensor_scalars on GpSimd, and affine prep (a*x+b) plus
  logs/squares go to the Act engine.
- All 3 input fields and both dF fields are SBUF-resident; inputs stream
  in as 2 column-chunks per field (>=512B descriptors, full DMA rate)
  overlapped with compute; outputs accumulate in 256-col staging tiles
  and store every second band (512B descriptors, no small-DMA penalty).
- Band temps are double-buffered (bufs=2) so consecutive 128-col bands
  pipeline across DVE/Act/GpSimd.
- Scalar parameters are baked as immediates; the program cache is keyed
  on their values (a param change only costs host compile time).

Layout per core: partition p = h // 8 (128 partitions), free dims =
(s = h % 8, w).  h+-1 stencil reads are free-dim shifts except at
s-block edges, which read halo row tiles ([P, WP]): pass-1 halos are
host-marshalled, pass-2 (dF) halos come from SBUF->SBUF DMA.
"""

import json
import math
import numpy as np
import ml_dtypes

import concourse.bass as bass
import concourse.mybir as mybir
from concourse.tile import TileContext
from concourse.bass_utils import run_bass_kernel_spmd

AF = mybir.ActivationFunctionType
OP = mybir.AluOpType
F32 = mybir.dt.float32
BF16 = mybir.dt.bfloat16
BF16_NP = ml_dtypes.bfloat16

# ---------------------------------------------------------------------------
# Workaround: this container's walrus accepts at most ONE sync wait per
# instruction; Tile merges several.  Split extras onto single-wait Drains.
# ---------------------------------------------------------------------------
def _split_waits_json(bj: bytes) -> bytes:
    m = json.loads(bj)
    for f in m["functions"]:
        for blk in f["blocks"]:
            out = []
            for ins in blk["instructions"]:
                si = ins.get("sync_info")
                waits = (si or {}).get("on_wait") or []
                if len(waits) > 1:
                    for k, w in enumerate(waits[:-1]):
                        out.append({
                            "debug": ins.get("debug", 0),
                            "engine": ins["engine"], "ins": [], "outs": [],
                            "is_reset_sema": False,
                            "name": f"{ins['name']}-wsplit{k}",
                            "opcode": "Drain",
                            "sync_info": {"on_update": [], "on_wait": [w]},
                        })
                    si["on_wait"] = [waits[-1]]
                out.append(ins)
            blk["instructions"] = out
    return json.dumps(m).encode()


if not getattr(bass.Bass, "_wait_split_patched", False):
    _orig_to_json_bytes = bass.Bass.to_json_bytes

    def _patched_to_json_bytes(self) -> bytes:
        return _split_waits_json(_orig_to_json_bytes(self))

    bass.Bass.to_json_bytes = _patched_to_json_bytes
    bass.Bass._wait_split_patched = True

# ---------------------------------------------------------------------------
# Problem constants
# ---------------------------------------------------------------------------
B, H, W = 8, 1024, 1024
P, S = 128, 8          # H = P * S
WP = W + 2             # w-padded width (halo cols)
WB = 128               # band width
NB = W // WB
EPS = 1e-6
DT = 1e-2
# input column chunks (padded coords), sized so each DMA descriptor >= 512B
CHUNKS = [(0, 514), (514, 1026)]


def build_nc(ev, ei, kT, kv, ki, ke, Dv, Di, L):
    g = DT * L
    a0 = 1.0 - 4.0 * g * ke     # eta_new = a0*eta + 2g*(fs-u) + g*ke*nsE
    bv = DT * Dv / kT
    bi = DT * Di / kT
    rt2 = float(math.sqrt(2.0))

    nc = bass.Bass()
    # register const APs needed as activation biases (Identity/Ln/Square)
    for cval in (float(ev), float(ei), -1.0):
        if (F32, cval) not in nc.const_aps.aps:
            t = nc.alloc_sbuf_tensor(f"constx-{cval}", [128, 1], F32)
            nc.gpsimd.memset(t.ap(), cval)
            nc.const_aps.aps[(F32, cval)] = t.ap()
    nc.all_engine_barrier()
    dp = nc.declare_dram_parameter
    cvp = dp("cvp", [H, WP], BF16, isOutput=False)
    cip = dp("cip", [H, WP], BF16, isOutput=False)
    etp = dp("etp", [H, WP], BF16, isOutput=False)
    smx = dp("smx", [H, W], BF16, isOutput=False)
    # row-halo arrays: row (8p-1)%1024 ("u") and (8p+8)%1024 ("d"), w-padded
    cvu = dp("cvu", [P, WP], BF16, isOutput=False)
    cvd = dp("cvd", [P, WP], BF16, isOutput=False)
    ciu = dp("ciu", [P, WP], BF16, isOutput=False)
    cid = dp("cid", [P, WP], BF16, isOutput=False)
    etu = dp("etu", [P, WP], BF16, isOutput=False)
    etd = dp("etd", [P, WP], BF16, isOutput=False)
    ocv = dp("cv_new", [H, W], BF16, isOutput=True)
    oci = dp("ci_new", [H, W], BF16, isOutput=True)
    oet = dp("eta_new", [H, W], BF16, isOutput=True)

    cvp3, cip3, etp3 = (x.rearrange("(p s) w -> p s w", s=S) for x in (cvp, cip, etp))
    smx3 = smx.rearrange("(p s) w -> p s w", s=S)
    ocv3, oci3, oet3 = (x.rearrange("(p s) w -> p s w", s=S) for x in (ocv, oci, oet))

    nv, ng, na = nc.vector, nc.gpsimd, nc.scalar

    with TileContext(nc) as tc:
        with tc.tile_pool(name="res", bufs=1) as res:
            # resident input fields and dF fields
            cvm = res.tile([P, S, WP], BF16)
            cim = res.tile([P, S, WP], BF16)
            eta = res.tile([P, S, WP], BF16)
            dFv = res.tile([P, S, WP], BF16)
            dFi = res.tile([P, S, WP], BF16)
            # interleave chunk loads across fields so band-0 deps land first
            for lo, hi in CHUNKS:
                nc.sync.dma_start(out=cvm[:, :, lo:hi], in_=cvp3[:, :, lo:hi])
                nc.sync.dma_start(out=cim[:, :, lo:hi], in_=cip3[:, :, lo:hi])
                nc.sync.dma_start(out=eta[:, :, lo:hi], in_=etp3[:, :, lo:hi])

            # ---------------- pass 1 ----------------
            with tc.tile_pool(name="p1", bufs=1) as p1:
                hcv_u = p1.tile([P, WP], BF16, tag="hcvu")
                hcv_d = p1.tile([P, WP], BF16, tag="hcvd")
                hci_u = p1.tile([P, WP], BF16, tag="hciu")
                hci_d = p1.tile([P, WP], BF16, tag="hcid")
                het_u = p1.tile([P, WP], BF16, tag="hetu")
                het_d = p1.tile([P, WP], BF16, tag="hetd")
                for t, src in ((hcv_u, cvu), (hcv_d, cvd), (hci_u, ciu),
                               (hci_d, cid), (het_u, etu), (het_d, etd)):
                    nc.sync.dma_start(out=t[:], in_=src[:])

                with tc.tile_pool(name="p1b", bufs=2) as p1b:
                    def T(tag, dt=BF16):
                        return p1b.tile([P, S, WB], dt, tag=tag, name=tag)

                    smb = oeb = None
                    for b in range(NB):
                        w0 = b * WB          # image col of band start
                        hs = slice(1 + w0, 1 + w0 + WB)   # padded interior cols

                        # S loads and eta_new stores batched 2 bands per DMA
                        # (512B descriptors, full DMA rate)
                        half = (b % 2) * WB
                        if b % 2 == 0:
                            smb = p1b.tile([P, S, 2 * WB], BF16, tag="smb")
                            nc.sync.dma_start(out=smb[:],
                                              in_=smx3[:, :, w0:w0 + 2 * WB])
                            oeb = p1b.tile([P, S, 2 * WB], BF16, tag="oeb")
                        sms = smb[:, :, half:half + WB]

                        def nsum(eng, dst_lr, dst_ud, ft, hu, hd):
                            # dst_lr = left+right ; dst_ud = up+down (band cols)
                            eng.tensor_tensor(dst_lr[:], ft[:, :, w0:w0 + WB],
                                              ft[:, :, w0 + 2:w0 + WB + 2], OP.add)
                            eng.tensor_tensor(dst_ud[:, 0, :], hu[:, hs],
                                              ft[:, 1, hs], OP.add)
                            eng.tensor_tensor(dst_ud[:, 1:7, :], ft[:, 0:6, hs],
                                              ft[:, 2:8, hs], OP.add)
                            eng.tensor_tensor(dst_ud[:, 7, :], ft[:, 6, hs],
                                              hd[:, hs], OP.add)

                        # Act block: logs (fp32 out) then klv/kli (bf16),
                        # then squares (grouped by activation function)
                        ls = T("ls", F32)
                        lv = p1b.tile([P, S, WB], F32, tag="lv", name="lv", bufs=1)
                        li = p1b.tile([P, S, WB], F32, tag="li", name="li", bufs=1)
                        na.activation(ls[:], sms, AF.Ln, bias=0.0, scale=1.0)
                        na.activation(lv[:], cvm[:, :, hs], AF.Ln, bias=0.0, scale=1.0)
                        na.activation(li[:], cim[:, :, hs], AF.Ln, bias=0.0, scale=1.0)
                        # klv = kT*lv + Ev ; kli = kT*li + Ei  (bf16 out)
                        klv, kli = T("klv"), T("kli")
                        na.activation(klv[:], lv[:], AF.Identity, bias=float(ev),
                                      scale=kT)
                        na.activation(kli[:], li[:], AF.Identity, bias=float(ei),
                                      scale=kT)
                        h, j2, b2, fv = T("h"), T("j2"), T("b2"), T("fv")
                        na.activation(h[:], eta[:, :, hs], AF.Square, bias=1.0, scale=-1.0)
                        na.activation(j2[:], eta[:, :, hs], AF.Square, bias=0.0, scale=rt2)
                        na.activation(b2[:], cim[:, :, hs], AF.Square, bias=0.0, scale=1.0)
                        na.activation(fv[:], cvm[:, :, hs], AF.Square, bias=1.0, scale=-1.0)
                        # affine prep on Act
                        e0, cm1 = T("e0"), T("cm1")
                        na.activation(e0[:], eta[:, :, hs], AF.Copy, bias=0.0,
                                      scale=a0)
                        na.activation(cm1[:], cvm[:, :, hs], AF.Identity,
                                      bias=-1.0, scale=1.0)

                        # klsp = kT*ls (DVE fp32 TS, bf16 out)
                        klsp = T("klsp")
                        nv.tensor_scalar(klsp[:], ls[:], kT, None, OP.mult)
                        # Pv = klv - klsp ; Pi = kli - klsp  (in place over klv/kli)
                        nv.tensor_tensor(klv[:], klv[:], klsp[:], OP.subtract)
                        nv.tensor_tensor(kli[:], kli[:], klsp[:], OP.subtract)
                        pv, pi = klv, kli

                        # neighbor sums (UD of eta on Pool, rest on DVE)
                        lr, ud, nse = T("lr"), T("ud"), T("nse")
                        nsum(nv, lr, ud, eta, het_u, het_d)
                        nv.tensor_tensor(nse[:], lr[:], ud[:], OP.add)
                        nsv, nsi = T("nsv"), T("nsi")
                        nsum(nv, lr, ud, cvm, hcv_u, hcv_d)
                        nv.tensor_tensor(nsv[:], lr[:], ud[:], OP.add)
                        nsum(nv, lr, ud, cim, hci_u, hci_d)
                        nv.tensor_tensor(nsi[:], lr[:], ud[:], OP.add)

                        # fs = cv*Pv + ci*Pi + klsp
                        f1, f2 = T("f1"), T("f2")
                        nv.tensor_tensor(f1[:], cvm[:, :, hs], pv[:], OP.mult)
                        nv.tensor_tensor(f2[:], cim[:, :, hs], pi[:], OP.mult)
                        nv.tensor_tensor(f1[:], f1[:], f2[:], OP.add)
                        fs = T("fs")
                        nv.tensor_tensor(fs[:], f1[:], klsp[:], OP.add)

                        # fv = (cv-1)^2 + ci^2 ; u = eta*(fs+fv) ; w = fs-u
                        nv.tensor_tensor(fv[:], fv[:], b2[:], OP.add)
                        nv.tensor_tensor(fv[:], fs[:], fv[:], OP.add)
                        nv.tensor_tensor(fv[:], eta[:, :, hs], fv[:], OP.mult)
                        nv.tensor_tensor(fv[:], fs[:], fv[:], OP.subtract)
                        w = fv

                        # eta_new = clip(a0*eta + 2g*w + g*ke*nsE)
                        # scale pieces on Pool (tensor_scalar), adds on DVE
                        ng.tensor_scalar(w[:], w[:], 2.0 * g, None, OP.mult)
                        ng.tensor_scalar(nse[:], nse[:], g * ke, None, OP.mult)
                        nv.tensor_tensor(e0[:], e0[:], w[:], OP.add)
                        nv.tensor_tensor(e0[:], e0[:], nse[:], OP.add)
                        ng.tensor_scalar(oeb[:, :, half:half + WB], e0[:],
                                         0.0, 1.0, OP.max, OP.min)
                        if b % 2 == 1:
                            nc.sync.dma_start(out=oet3[:, :, w0 - WB:w0 + WB],
                                              in_=oeb[:])

                        # dFv = h*Pv + j2*(cv-1) - kv*nsv + 4kv*cv
                        mv, nv_ = T("mv"), T("nv_")
                        nv.tensor_tensor(mv[:], h[:], pv[:], OP.mult)
                        nv.tensor_tensor(nv_[:], cm1[:], j2[:], OP.mult)
                        nv.tensor_tensor(mv[:], mv[:], nv_[:], OP.add)
                        ng.tensor_scalar(nsv[:], nsv[:], -kv, None, OP.mult)
                        s4 = T("s4")
                        ng.tensor_scalar(s4[:], cvm[:, :, hs], 4.0 * kv, None,
                                         OP.mult)
                        nv.tensor_tensor(mv[:], mv[:], nsv[:], OP.add)
                        nv.tensor_tensor(dFv[:, :, hs], mv[:], s4[:], OP.add)

                        # dFi = h*Pi + j2*ci - ki*nsi + 4ki*ci
                        mi, ni_ = T("mv"), T("nv_")
                        nv.tensor_tensor(mi[:], h[:], pi[:], OP.mult)
                        nv.tensor_tensor(ni_[:], cim[:, :, hs], j2[:], OP.mult)
                        nv.tensor_tensor(mi[:], mi[:], ni_[:], OP.add)
                        ng.tensor_scalar(nsi[:], nsi[:], -ki, None, OP.mult)
                        s4b = T("s4")
                        ng.tensor_scalar(s4b[:], cim[:, :, hs], 4.0 * ki, None,
                                         OP.mult)
                        nv.tensor_tensor(mi[:], mi[:], nsi[:], OP.add)
                        nv.tensor_tensor(dFi[:, :, hs], mi[:], s4b[:], OP.add)

            # ---------------- dF halo fill ----------------
            with tc.tile_pool(name="p2", bufs=1) as p2:
                for t in (dFv, dFi):
                    nv.tensor_copy(t[:, :, 0:1], t[:, :, W:W + 1])
                    nv.tensor_copy(t[:, :, W + 1:W + 2], t[:, :, 1:2])
                hv_u = p2.tile([P, WP], BF16, tag="hvu")
                hv_d = p2.tile([P, WP], BF16, tag="hvd")
                hi_u = p2.tile([P, WP], BF16, tag="hiu")
                hi_d = p2.tile([P, WP], BF16, tag="hid")
                for src, hu, hd in ((dFv, hv_u, hv_d), (dFi, hi_u, hi_d)):
                    nc.sync.dma_start(out=hu[1:P, :], in_=src[0:P - 1, 7, :])
                    nc.sync.dma_start(out=hu[0:1, :], in_=src[P - 1:P, 7, :])
                    nc.sync.dma_start(out=hd[0:P - 1, :], in_=src[1:P, 0, :])
                    nc.sync.dma_start(out=hd[P - 1:P, :], in_=src[0:1, 0, :])

                # ---------------- pass 2 ----------------
                with tc.tile_pool(name="p2b", bufs=2) as p2b:
                    def T2(tag):
                        return p2b.tile([P, S, WB], BF16, tag=tag, name=tag)

                    obs = {}
                    for b in range(NB):
                        w0 = b * WB
                        hs = slice(1 + w0, 1 + w0 + WB)
                        half = (b % 2) * WB
                        if b % 2 == 0:
                            obs["obv"] = p2b.tile([P, S, 2 * WB], BF16, tag="obv", name="obv")
                            obs["obi"] = p2b.tile([P, S, 2 * WB], BF16, tag="obi", name="obi")
                        lr, ud, ns = T2("l2"), T2("u2"), T2("n2")
                        t1f = p2b.tile([P, S, WB], F32, tag="t1f", name="t1f")
                        t2f = p2b.tile([P, S, WB], F32, tag="t2f", name="t2f")

                        def lap2(dF, hu, hd, cres, beta, dst_dram, obtag):
                            nv.tensor_tensor(lr[:], dF[:, :, w0:w0 + WB],
                                             dF[:, :, w0 + 2:w0 + WB + 2], OP.add)
                            nv.tensor_tensor(ud[:, 0, :], hu[:, hs], dF[:, 1, hs], OP.add)
                            nv.tensor_tensor(ud[:, 1:7, :], dF[:, 0:6, hs],
                                             dF[:, 2:8, hs], OP.add)
                            nv.tensor_tensor(ud[:, 7, :], dF[:, 6, hs], hd[:, hs], OP.add)
                            nv.tensor_tensor(ns[:], lr[:], ud[:], OP.add)
                            # new = clip(c * (1 + beta*(ns - 4*dF)))
                            # t1/t2/m/pm in fp32 for precision (cheap: Act and
                            # Pool ops are dtype-free, only 2 DVE ops go 1x)
                            na.activation(t1f[:], ns[:], AF.Identity, bias=1.0,
                                          scale=beta)
                            ng.tensor_scalar(t2f[:], dF[:, :, hs], -4.0 * beta,
                                             None, OP.mult)
                            nv.tensor_tensor(t1f[:], t1f[:], t2f[:], OP.add)
                            nv.tensor_tensor(t1f[:], cres[:, :, hs], t1f[:], OP.mult)
                            ob = obs[obtag]
                            ng.tensor_scalar(ob[:, :, half:half + WB], t1f[:],
                                             0.0, 1.0, OP.max, OP.min)
                            if b % 2 == 1:
                                nc.sync.dma_start(out=dst_dram[:, :, w0 - WB:w0 + WB],
                                                  in_=ob[:])

                        lap2(dFv, hv_u, hv_d, cvm, bv, ocv3, "obv")
                        lap2(dFi, hi_u, hi_d, cim, bi, oci3, "obi")
    return nc


_NC_CACHE = {}


def _get_nc(params=None):
    global _NC_CACHE
    if params is None:
        if _NC_CACHE:
            return next(iter(_NC_CACHE.values()))
        params = (0.7729, 0.5245, 0.2182, 0.6689, 0.1679, 0.2640,
                  0.7368, 0.8902, 0.1332)
    key = tuple(round(float(x), 9) for x in params)
    if key not in _NC_CACHE:
        _NC_CACHE[key] = build_nc(*params)
    return _NC_CACHE[key]


def _pad_w(x):
    out = np.empty((x.shape[0], WP), x.dtype)
    out[:, 1:W + 1] = x
    out[:, 0] = x[:, W - 1]
    out[:, W + 1] = x[:, 0]
    return out


_IDX_U = (np.arange(P) * S - 1) % H
_IDX_D = (np.arange(P) * S + S) % H


def kernel(cv, ci, eta, energy_v0, energy_i0, kBT0, kappa_v0, kappa_i0,
           kappa_eta0, diff_v0, diff_i0, L0):
    cv = np.asarray(cv, np.float32)
    ci = np.asarray(ci, np.float32)
    eta = np.asarray(eta, np.float32)
    ab = lambda v: abs(float(np.asarray(v).reshape(-1)[0])) + 0.001
    ev, ei, kT = ab(energy_v0), ab(energy_i0), ab(kBT0)
    kv, ki, ke = ab(kappa_v0), ab(kappa_i0), ab(kappa_eta0)
    Dv, Di, L = ab(diff_v0), ab(diff_i0), ab(L0)

    in_maps = []
    for i in range(B):
        cvm = np.maximum(cv[i], EPS).astype(BF16_NP)
        cim = np.maximum(ci[i], EPS).astype(BF16_NP)
        et16 = eta[i].astype(BF16_NP)
        smx = np.maximum(1.0 - cv[i] - ci[i], EPS).astype(BF16_NP)
        cvp, cip, etp = _pad_w(cvm), _pad_w(cim), _pad_w(et16)
        in_maps.append({
            "cvp": cvp, "cip": cip, "etp": etp, "smx": smx,
            "cvu": np.ascontiguousarray(cvp[_IDX_U]),
            "cvd": np.ascontiguousarray(cvp[_IDX_D]),
            "ciu": np.ascontiguousarray(cip[_IDX_U]),
            "cid": np.ascontiguousarray(cip[_IDX_D]),
            "etu": np.ascontiguousarray(etp[_IDX_U]),
            "etd": np.ascontiguousarray(etp[_IDX_D]),
        })

    nc = _get_nc((ev, ei, kT, kv, ki, ke, Dv, Di, L))
    res = run_bass_kernel_spmd(nc, in_maps, core_ids=list(range(B)))
    cv_new = np.stack([r["cv_new"] for r in res.results]).astype(np.float32)
    ci_new = np.stack([r["ci_new"] for r in res.results]).astype(np.float32)
    eta_new = np.stack([r["eta_new"] for r in res.results]).astype(np.float32)
    return cv_new, ci_new, eta_new


# revision 29
# speedup vs baseline: 1.0384x; 1.0384x over previous
"""Trainium2 Bass kernel for nn_IrradiationSingleTimestep.

Phase-field irradiation single timestep: 3 fields (cv, ci, eta) of shape
[8, 1024, 1024], 5-point periodic Laplacians (two levels), pointwise
thermodynamics with logs, clipped Euler update.

Sharding: batch-parallel, one batch image per NeuronCore (8 cores).

Strategy (bf16 compute, engine-balanced):
- Host precomputes max(cv,eps), max(ci,eps), max(1-cv-ci,eps) in fp32 and
  rounds to bf16.  The 1-cv-ci cancellation is the only absolute-error
  amplifier through ln(); doing it on host keeps ln() errors relative
  (~2^-9) everywhere.  Logs run on the Act engine with fp32 outputs;
  the pass-2 update chain is fp32 for output-error margin.
- DVE perf modes: tensor_scalar bf16 packed = 4x, tensor_tensor bf16
  packed = 2x, fp32 all-SBUF tensor_scalar = 2x, scalar_tensor_tensor =
  no fast mode (and ILLEGAL on GpSimd in the real ISA).  So tensor x
  tensor ops are plain bf16 tensor_tensors on DVE, standalone scalings
  and clips are tensor_scalars on GpSimd, and affine prep (a*x+b) plus
  logs/squares go to the Act engine.
- All 3 input fields and both dF fields are SBUF-resident; inputs stream
  in as 2 column-chunks per field (>=512B descriptors, full DMA rate)
  overlapped with compute; outputs accumulate in 256-col staging tiles
  and store every second band (512B descriptors, no small-DMA penalty).
- Band temps are double-buffered (bufs=2) so consecutive 128-col bands
  pipeline across DVE/Act/GpSimd.
- Scalar parameters are baked as immediates; the program cache is keyed
  on their values (a param change only costs host compile time).

Layout per core: partition p = h // 8 (128 partitions), free dims =
(s = h % 8, w).  h+-1 stencil reads are free-dim shifts except at
s-block edges, which read halo row tiles ([P, WP]): pass-1 halos are
host-marshalled, pass-2 (dF) halos come from SBUF->SBUF DMA.
"""

import json
import math
import numpy as np
import ml_dtypes

import concourse.bass as bass
import concourse.mybir as mybir
from concourse.tile import TileContext
from concourse.bass_utils import run_bass_kernel_spmd

AF = mybir.ActivationFunctionType
OP = mybir.AluOpType
F32 = mybir.dt.float32
BF16 = mybir.dt.bfloat16
BF16_NP = ml_dtypes.bfloat16

# ---------------------------------------------------------------------------
# Workaround: this container's walrus accepts at most ONE sync wait per
# instruction; Tile merges several.  Split extras onto single-wait Drains.
# ---------------------------------------------------------------------------
def _split_waits_json(bj: bytes) -> bytes:
    m = json.loads(bj)
    for f in m["functions"]:
        for blk in f["blocks"]:
            out = []
            for ins in blk["instructions"]:
                si = ins.get("sync_info")
                waits = (si or {}).get("on_wait") or []
                if len(waits) > 1:
                    for k, w in enumerate(waits[:-1]):
                        out.append({
                            "debug": ins.get("debug", 0),
                            "engine": ins["engine"], "ins": [], "outs": [],
                            "is_reset_sema": False,
                            "name": f"{ins['name']}-wsplit{k}",
                            "opcode": "Drain",
                            "sync_info": {"on_update": [], "on_wait": [w]},
                        })
                    si["on_wait"] = [waits[-1]]
                out.append(ins)
            blk["instructions"] = out
    return json.dumps(m).encode()


if not getattr(bass.Bass, "_wait_split_patched", False):
    _orig_to_json_bytes = bass.Bass.to_json_bytes

    def _patched_to_json_bytes(self) -> bytes:
        return _split_waits_json(_orig_to_json_bytes(self))

    bass.Bass.to_json_bytes = _patched_to_json_bytes
    bass.Bass._wait_split_patched = True

# ---------------------------------------------------------------------------
# Problem constants
# ---------------------------------------------------------------------------
B, H, W = 8, 1024, 1024
P, S = 128, 8          # H = P * S
WP = W + 2             # w-padded width (halo cols)
WB = 128               # band width
NB = W // WB
EPS = 1e-6
DT = 1e-2
# input column chunks (padded coords), sized so each DMA descriptor >= 512B
CHUNKS = [(0, 258), (258, 514), (514, 770), (770, 1026)]


def build_nc(ev, ei, kT, kv, ki, ke, Dv, Di, L):
    g = DT * L
    a0 = 1.0 - 4.0 * g * ke     # eta_new = a0*eta + 2g*(fs-u) + g*ke*nsE
    bv = DT * Dv / kT
    bi = DT * Di / kT
    rt2 = float(math.sqrt(2.0))

    nc = bass.Bass()
    # register const APs needed as activation biases (Identity/Ln/Square)
    for cval in (float(ev), float(ei), -1.0):
        if (F32, cval) not in nc.const_aps.aps:
            t = nc.alloc_sbuf_tensor(f"constx-{cval}", [128, 1], F32)
            nc.gpsimd.memset(t.ap(), cval)
            nc.const_aps.aps[(F32, cval)] = t.ap()
    nc.all_engine_barrier()
    dp = nc.declare_dram_parameter
    cvp = dp("cvp", [H, WP], BF16, isOutput=False)
    cip = dp("cip", [H, WP], BF16, isOutput=False)
    etp = dp("etp", [H, WP], BF16, isOutput=False)
    smx = dp("smx", [H, W], BF16, isOutput=False)
    # row-halo arrays: row (8p-1)%1024 ("u") and (8p+8)%1024 ("d"), w-padded
    cvu = dp("cvu", [P, WP], BF16, isOutput=False)
    cvd = dp("cvd", [P, WP], BF16, isOutput=False)
    ciu = dp("ciu", [P, WP], BF16, isOutput=False)
    cid = dp("cid", [P, WP], BF16, isOutput=False)
    etu = dp("etu", [P, WP], BF16, isOutput=False)
    etd = dp("etd", [P, WP], BF16, isOutput=False)
    ocv = dp("cv_new", [H, W], BF16, isOutput=True)
    oci = dp("ci_new", [H, W], BF16, isOutput=True)
    oet = dp("eta_new", [H, W], BF16, isOutput=True)

    cvp3, cip3, etp3 = (x.rearrange("(p s) w -> p s w", s=S) for x in (cvp, cip, etp))
    smx3 = smx.rearrange("(p s) w -> p s w", s=S)
    ocv3, oci3, oet3 = (x.rearrange("(p s) w -> p s w", s=S) for x in (ocv, oci, oet))

    nv, ng, na = nc.vector, nc.gpsimd, nc.scalar

    with TileContext(nc) as tc:
        with tc.tile_pool(name="res", bufs=1) as res:
            # resident input fields and dF fields
            cvm = res.tile([P, S, WP], BF16)
            cim = res.tile([P, S, WP], BF16)
            eta = res.tile([P, S, WP], BF16)
            dFv = res.tile([P, S, WP], BF16)
            dFi = res.tile([P, S, WP], BF16)
            # interleave chunk loads across fields so band-0 deps land first
            for lo, hi in CHUNKS:
                nc.sync.dma_start(out=cvm[:, :, lo:hi], in_=cvp3[:, :, lo:hi])
                nc.sync.dma_start(out=cim[:, :, lo:hi], in_=cip3[:, :, lo:hi])
                nc.sync.dma_start(out=eta[:, :, lo:hi], in_=etp3[:, :, lo:hi])

            # ---------------- pass 1 ----------------
            with tc.tile_pool(name="p1", bufs=1) as p1:
                hcv_u = p1.tile([P, WP], BF16, tag="hcvu")
                hcv_d = p1.tile([P, WP], BF16, tag="hcvd")
                hci_u = p1.tile([P, WP], BF16, tag="hciu")
                hci_d = p1.tile([P, WP], BF16, tag="hcid")
                het_u = p1.tile([P, WP], BF16, tag="hetu")
                het_d = p1.tile([P, WP], BF16, tag="hetd")
                for t, src in ((hcv_u, cvu), (hcv_d, cvd), (hci_u, ciu),
                               (hci_d, cid), (het_u, etu), (het_d, etd)):
                    nc.sync.dma_start(out=t[:], in_=src[:])

                with tc.tile_pool(name="p1b", bufs=2) as p1b:
                    def T(tag, dt=BF16):
                        return p1b.tile([P, S, WB], dt, tag=tag, name=tag)

                    smb = oeb = None
                    for b in range(NB):
                        w0 = b * WB          # image col of band start
                        hs = slice(1 + w0, 1 + w0 + WB)   # padded interior cols

                        # S loads and eta_new stores batched 2 bands per DMA
                        # (512B descriptors, full DMA rate)
                        half = (b % 2) * WB
                        if b % 2 == 0:
                            smb = p1b.tile([P, S, 2 * WB], BF16, tag="smb")
                            nc.sync.dma_start(out=smb[:],
                                              in_=smx3[:, :, w0:w0 + 2 * WB])
                            oeb = p1b.tile([P, S, 2 * WB], BF16, tag="oeb")
                        sms = smb[:, :, half:half + WB]

                        def nsum(eng, dst_lr, dst_ud, ft, hu, hd):
                            # dst_lr = left+right ; dst_ud = up+down (band cols)
                            eng.tensor_tensor(dst_lr[:], ft[:, :, w0:w0 + WB],
                                              ft[:, :, w0 + 2:w0 + WB + 2], OP.add)
                            eng.tensor_tensor(dst_ud[:, 0, :], hu[:, hs],
                                              ft[:, 1, hs], OP.add)
                            eng.tensor_tensor(dst_ud[:, 1:7, :], ft[:, 0:6, hs],
                                              ft[:, 2:8, hs], OP.add)
                            eng.tensor_tensor(dst_ud[:, 7, :], ft[:, 6, hs],
                                              hd[:, hs], OP.add)

                        # Act block: logs (fp32 out) then klv/kli (bf16),
                        # then squares (grouped by activation function)
                        ls = T("ls", F32)
                        lv = p1b.tile([P, S, WB], F32, tag="lv", name="lv", bufs=1)
                        li = p1b.tile([P, S, WB], F32, tag="li", name="li", bufs=1)
                        na.activation(ls[:], sms, AF.Ln, bias=0.0, scale=1.0)
                        na.activation(lv[:], cvm[:, :, hs], AF.Ln, bias=0.0, scale=1.0)
                        na.activation(li[:], cim[:, :, hs], AF.Ln, bias=0.0, scale=1.0)
                        # klv = kT*lv + Ev ; kli = kT*li + Ei  (bf16 out)
                        klv, kli = T("klv"), T("kli")
                        na.activation(klv[:], lv[:], AF.Identity, bias=float(ev),
                                      scale=kT)
                        na.activation(kli[:], li[:], AF.Identity, bias=float(ei),
                                      scale=kT)
                        h, j2, b2, fv = T("h"), T("j2"), T("b2"), T("fv")
                        na.activation(h[:], eta[:, :, hs], AF.Square, bias=1.0, scale=-1.0)
                        na.activation(j2[:], eta[:, :, hs], AF.Square, bias=0.0, scale=rt2)
                        na.activation(b2[:], cim[:, :, hs], AF.Square, bias=0.0, scale=1.0)
                        na.activation(fv[:], cvm[:, :, hs], AF.Square, bias=1.0, scale=-1.0)
                        # affine prep on Act
                        e0, cm1 = T("e0"), T("cm1")
                        na.activation(e0[:], eta[:, :, hs], AF.Copy, bias=0.0,
                                      scale=a0)
                        na.activation(cm1[:], cvm[:, :, hs], AF.Identity,
                                      bias=-1.0, scale=1.0)

                        # klsp = kT*ls (Act, bf16 out)
                        klsp = T("klsp")
                        na.activation(klsp[:], ls[:], AF.Copy, bias=0.0, scale=kT)
                        # Pv = klv - klsp ; Pi = kli - klsp  (in place over klv/kli)
                        nv.tensor_tensor(klv[:], klv[:], klsp[:], OP.subtract)
                        nv.tensor_tensor(kli[:], kli[:], klsp[:], OP.subtract)
                        pv, pi = klv, kli

                        # neighbor sums (UD of eta on Pool, rest on DVE)
                        lr, ud, nse = T("lr"), T("ud"), T("nse")
                        nsum(nv, lr, ud, eta, het_u, het_d)
                        nv.tensor_tensor(nse[:], lr[:], ud[:], OP.add)
                        nsv, nsi = T("nsv"), T("nsi")
                        nsum(nv, lr, ud, cvm, hcv_u, hcv_d)
                        nv.tensor_tensor(nsv[:], lr[:], ud[:], OP.add)
                        nsum(nv, lr, ud, cim, hci_u, hci_d)
                        nv.tensor_tensor(nsi[:], lr[:], ud[:], OP.add)

                        # fs = cv*Pv + ci*Pi + klsp
                        f1, f2 = T("f1"), T("f2")
                        nv.tensor_tensor(f1[:], cvm[:, :, hs], pv[:], OP.mult)
                        nv.tensor_tensor(f2[:], cim[:, :, hs], pi[:], OP.mult)
                        nv.tensor_tensor(f1[:], f1[:], f2[:], OP.add)
                        fs = T("fs")
                        nv.tensor_tensor(fs[:], f1[:], klsp[:], OP.add)

                        # fv = (cv-1)^2 + ci^2 ; u = eta*(fs+fv) ; w = fs-u
                        nv.tensor_tensor(fv[:], fv[:], b2[:], OP.add)
                        nv.tensor_tensor(fv[:], fs[:], fv[:], OP.add)
                        nv.tensor_tensor(fv[:], eta[:, :, hs], fv[:], OP.mult)
                        nv.tensor_tensor(fv[:], fs[:], fv[:], OP.subtract)
                        w = fv

                        # eta_new = clip(a0*eta + 2g*w + g*ke*nsE)
                        # scale pieces on Pool (tensor_scalar), adds on DVE
                        ng.tensor_scalar(w[:], w[:], 2.0 * g, None, OP.mult)
                        ng.tensor_scalar(nse[:], nse[:], g * ke, None, OP.mult)
                        nv.tensor_tensor(e0[:], e0[:], w[:], OP.add)
                        nv.tensor_tensor(e0[:], e0[:], nse[:], OP.add)
                        ng.tensor_scalar(oeb[:, :, half:half + WB], e0[:],
                                         0.0, 1.0, OP.max, OP.min)
                        if b % 2 == 1:
                            nc.sync.dma_start(out=oet3[:, :, w0 - WB:w0 + WB],
                                              in_=oeb[:])

                        # dFv = h*Pv + j2*(cv-1) - kv*nsv + 4kv*cv
                        mv, nv_ = T("mv"), T("nv_")
                        nv.tensor_tensor(mv[:], h[:], pv[:], OP.mult)
                        nv.tensor_tensor(nv_[:], cm1[:], j2[:], OP.mult)
                        nv.tensor_tensor(mv[:], mv[:], nv_[:], OP.add)
                        ng.tensor_scalar(nsv[:], nsv[:], -kv, None, OP.mult)
                        s4 = T("s4")
                        ng.tensor_scalar(s4[:], cvm[:, :, hs], 4.0 * kv, None,
                                         OP.mult)
                        nv.tensor_tensor(mv[:], mv[:], nsv[:], OP.add)
                        nv.tensor_tensor(dFv[:, :, hs], mv[:], s4[:], OP.add)

                        # dFi = h*Pi + j2*ci - ki*nsi + 4ki*ci
                        mi, ni_ = T("mv"), T("nv_")
                        nv.tensor_tensor(mi[:], h[:], pi[:], OP.mult)
                        nv.tensor_tensor(ni_[:], cim[:, :, hs], j2[:], OP.mult)
                        nv.tensor_tensor(mi[:], mi[:], ni_[:], OP.add)
                        ng.tensor_scalar(nsi[:], nsi[:], -ki, None, OP.mult)
                        s4b = T("s4")
                        ng.tensor_scalar(s4b[:], cim[:, :, hs], 4.0 * ki, None,
                                         OP.mult)
                        nv.tensor_tensor(mi[:], mi[:], nsi[:], OP.add)
                        nv.tensor_tensor(dFi[:, :, hs], mi[:], s4b[:], OP.add)

            # ---------------- dF halo fill ----------------
            with tc.tile_pool(name="p2", bufs=1) as p2:
                for t in (dFv, dFi):
                    nv.tensor_copy(t[:, :, 0:1], t[:, :, W:W + 1])
                    nv.tensor_copy(t[:, :, W + 1:W + 2], t[:, :, 1:2])
                hv_u = p2.tile([P, WP], BF16, tag="hvu")
                hv_d = p2.tile([P, WP], BF16, tag="hvd")
                hi_u = p2.tile([P, WP], BF16, tag="hiu")
                hi_d = p2.tile([P, WP], BF16, tag="hid")
                # row-halo DMAs read only interior cols and go in column
                # halves, so pass-2 band 0 isn't gated on the whole fill;
                # the halo tiles' wrap cols are filled by local copies.
                for lo, hi in ((1, 514), (514, W + 1)):
                    for src, hu, hd in ((dFv, hv_u, hv_d), (dFi, hi_u, hi_d)):
                        nc.sync.dma_start(out=hu[1:P, lo:hi],
                                          in_=src[0:P - 1, 7, lo:hi])
                        nc.sync.dma_start(out=hu[0:1, lo:hi],
                                          in_=src[P - 1:P, 7, lo:hi])
                        nc.sync.dma_start(out=hd[0:P - 1, lo:hi],
                                          in_=src[1:P, 0, lo:hi])
                        nc.sync.dma_start(out=hd[P - 1:P, lo:hi],
                                          in_=src[0:1, 0, lo:hi])
                for t in (hv_u, hv_d, hi_u, hi_d):
                    nv.tensor_copy(t[:, 0:1], t[:, W:W + 1])
                    nv.tensor_copy(t[:, W + 1:W + 2], t[:, 1:2])

                # ---------------- pass 2 ----------------
                with tc.tile_pool(name="p2b", bufs=2) as p2b:
                    def T2(tag):
                        return p2b.tile([P, S, WB], BF16, tag=tag, name=tag)

                    obs = {}
                    for b in range(NB):
                        w0 = b * WB
                        hs = slice(1 + w0, 1 + w0 + WB)
                        half = (b % 2) * WB
                        if b % 2 == 0:
                            obs["obv"] = p2b.tile([P, S, 2 * WB], BF16, tag="obv", name="obv")
                            obs["obi"] = p2b.tile([P, S, 2 * WB], BF16, tag="obi", name="obi")
                        tags = {}
                        for fz in ("v", "i"):
                            tags[fz] = (
                                T2("l2" + fz), T2("u2" + fz), T2("n2" + fz),
                                p2b.tile([P, S, WB], F32, tag="t1f" + fz, name="t1f"),
                                p2b.tile([P, S, WB], F32, tag="t2f" + fz, name="t2f"),
                            )

                        def lap2(dF, hu, hd, cres, beta, dst_dram, obtag, fz):
                            lr, ud, ns, t1f, t2f = tags[fz]
                            nv.tensor_tensor(lr[:], dF[:, :, w0:w0 + WB],
                                             dF[:, :, w0 + 2:w0 + WB + 2], OP.add)
                            nv.tensor_tensor(ud[:, 0, :], hu[:, hs], dF[:, 1, hs], OP.add)
                            nv.tensor_tensor(ud[:, 1:7, :], dF[:, 0:6, hs],
                                             dF[:, 2:8, hs], OP.add)
                            nv.tensor_tensor(ud[:, 7, :], dF[:, 6, hs], hd[:, hs], OP.add)
                            nv.tensor_tensor(ns[:], lr[:], ud[:], OP.add)
                            # new = clip(c * (1 + beta*(ns - 4*dF)))
                            # t1/t2/m/pm in fp32 for precision (cheap: Act and
                            # Pool ops are dtype-free, only 2 DVE ops go 1x)
                            na.activation(t1f[:], ns[:], AF.Identity, bias=1.0,
                                          scale=beta)
                            ng.tensor_scalar(t2f[:], dF[:, :, hs], -4.0 * beta,
                                             None, OP.mult)
                            nv.tensor_tensor(t1f[:], t1f[:], t2f[:], OP.add)
                            nv.tensor_tensor(t1f[:], cres[:, :, hs], t1f[:], OP.mult)
                            ob = obs[obtag]
                            ng.tensor_scalar(ob[:, :, half:half + WB], t1f[:],
                                             0.0, 1.0, OP.max, OP.min)
                            if b % 2 == 1:
                                nc.sync.dma_start(out=dst_dram[:, :, w0 - WB:w0 + WB],
                                                  in_=ob[:])

                        lap2(dFv, hv_u, hv_d, cvm, bv, ocv3, "obv", "v")
                        lap2(dFi, hi_u, hi_d, cim, bi, oci3, "obi", "i")
    return nc


_NC_CACHE = {}


def _get_nc(params=None):
    global _NC_CACHE
    if params is None:
        if _NC_CACHE:
            return next(iter(_NC_CACHE.values()))
        params = (0.7729, 0.5245, 0.2182, 0.6689, 0.1679, 0.2640,
                  0.7368, 0.8902, 0.1332)
    key = tuple(round(float(x), 9) for x in params)
    if key not in _NC_CACHE:
        _NC_CACHE[key] = build_nc(*params)
    return _NC_CACHE[key]


def _pad_w(x):
    out = np.empty((x.shape[0], WP), x.dtype)
    out[:, 1:W + 1] = x
    out[:, 0] = x[:, W - 1]
    out[:, W + 1] = x[:, 0]
    return out


_IDX_U = (np.arange(P) * S - 1) % H
_IDX_D = (np.arange(P) * S + S) % H


def kernel(cv, ci, eta, energy_v0, energy_i0, kBT0, kappa_v0, kappa_i0,
           kappa_eta0, diff_v0, diff_i0, L0):
    cv = np.asarray(cv, np.float32)
    ci = np.asarray(ci, np.float32)
    eta = np.asarray(eta, np.float32)
    ab = lambda v: abs(float(np.asarray(v).reshape(-1)[0])) + 0.001
    ev, ei, kT = ab(energy_v0), ab(energy_i0), ab(kBT0)
    kv, ki, ke = ab(kappa_v0), ab(kappa_i0), ab(kappa_eta0)
    Dv, Di, L = ab(diff_v0), ab(diff_i0), ab(L0)

    in_maps = []
    for i in range(B):
        cvm = np.maximum(cv[i], EPS).astype(BF16_NP)
        cim = np.maximum(ci[i], EPS).astype(BF16_NP)
        et16 = eta[i].astype(BF16_NP)
        smx = np.maximum(1.0 - cv[i] - ci[i], EPS).astype(BF16_NP)
        cvp, cip, etp = _pad_w(cvm), _pad_w(cim), _pad_w(et16)
        in_maps.append({
            "cvp": cvp, "cip": cip, "etp": etp, "smx": smx,
            "cvu": np.ascontiguousarray(cvp[_IDX_U]),
            "cvd": np.ascontiguousarray(cvp[_IDX_D]),
            "ciu": np.ascontiguousarray(cip[_IDX_U]),
            "cid": np.ascontiguousarray(cip[_IDX_D]),
            "etu": np.ascontiguousarray(etp[_IDX_U]),
            "etd": np.ascontiguousarray(etp[_IDX_D]),
        })

    nc = _get_nc((ev, ei, kT, kv, ki, ke, Dv, Di, L))
    res = run_bass_kernel_spmd(nc, in_maps, core_ids=list(range(B)))
    cv_new = np.stack([r["cv_new"] for r in res.results]).astype(np.float32)
    ci_new = np.stack([r["ci_new"] for r in res.results]).astype(np.float32)
    eta_new = np.stack([r["eta_new"] for r in res.results]).astype(np.float32)
    return cv_new, ci_new, eta_new


# revision 38
# speedup vs baseline: 1.0527x; 1.0138x over previous
"""Trainium2 Bass kernel for nn_IrradiationSingleTimestep.

Phase-field irradiation single timestep: 3 fields (cv, ci, eta) of shape
[8, 1024, 1024], 5-point periodic Laplacians (two levels), pointwise
thermodynamics with logs, clipped Euler update.

Sharding: batch-parallel, one batch image per NeuronCore (8 cores).

Strategy (bf16 compute, engine-balanced):
- Host precomputes max(cv,eps), max(ci,eps), max(1-cv-ci,eps) in fp32 and
  rounds to bf16.  The 1-cv-ci cancellation is the only absolute-error
  amplifier through ln(); doing it on host keeps ln() errors relative
  (~2^-9) everywhere.  Logs run on the Act engine with fp32 outputs;
  the pass-2 update chain is fp32 for output-error margin.
- DVE perf modes: tensor_scalar bf16 packed = 4x, tensor_tensor bf16
  packed = 2x, fp32 all-SBUF tensor_scalar = 2x, scalar_tensor_tensor =
  no fast mode (and ILLEGAL on GpSimd in the real ISA).  So tensor x
  tensor ops are plain bf16 tensor_tensors on DVE, standalone scalings
  and clips are tensor_scalars on GpSimd, and affine prep (a*x+b) plus
  logs/squares go to the Act engine.
- All 3 input fields and both dF fields are SBUF-resident; inputs stream
  in as 4 column-chunks per field (>=512B descriptors, full DMA rate)
  overlapped with compute; outputs accumulate in 256-col staging tiles
  and store every second band (512B descriptors, no small-DMA penalty).
- Band temps are double-buffered (bufs=2) so consecutive 128-col bands
  pipeline across DVE/Act/GpSimd.
- Scalar parameters are baked as immediates; the program cache is keyed
  on their values (a param change only costs host compile time).

Layout per core: partition p = h // 8 (128 partitions), free dims =
(s = h % 8, w).  h+-1 stencil reads are free-dim shifts except at
s-block edges, which read halo row tiles ([P, WP]): pass-1 halos are
host-marshalled, pass-2 (dF) halos come from SBUF->SBUF DMA.
"""

import json
import math
import numpy as np
import ml_dtypes

import concourse.bass as bass
import concourse.mybir as mybir
from concourse.tile import TileContext
from concourse.bass_utils import run_bass_kernel_spmd

AF = mybir.ActivationFunctionType
OP = mybir.AluOpType
F32 = mybir.dt.float32
BF16 = mybir.dt.bfloat16
BF16_NP = ml_dtypes.bfloat16

# ---------------------------------------------------------------------------
# Workaround: this container's walrus accepts at most ONE sync wait per
# instruction; Tile merges several.  Split extras onto single-wait Drains.
# ---------------------------------------------------------------------------
def _split_waits_json(bj: bytes) -> bytes:
    m = json.loads(bj)
    for f in m["functions"]:
        for blk in f["blocks"]:
            out = []
            for ins in blk["instructions"]:
                si = ins.get("sync_info")
                waits = (si or {}).get("on_wait") or []
                if len(waits) > 1:
                    for k, w in enumerate(waits[:-1]):
                        out.append({
                            "debug": ins.get("debug", 0),
                            "engine": ins["engine"], "ins": [], "outs": [],
                            "is_reset_sema": False,
                            "name": f"{ins['name']}-wsplit{k}",
                            "opcode": "Drain",
                            "sync_info": {"on_update": [], "on_wait": [w]},
                        })
                    si["on_wait"] = [waits[-1]]
                out.append(ins)
            blk["instructions"] = out
    return json.dumps(m).encode()


if not getattr(bass.Bass, "_wait_split_patched", False):
    _orig_to_json_bytes = bass.Bass.to_json_bytes

    def _patched_to_json_bytes(self) -> bytes:
        return _split_waits_json(_orig_to_json_bytes(self))

    bass.Bass.to_json_bytes = _patched_to_json_bytes
    bass.Bass._wait_split_patched = True

# ---------------------------------------------------------------------------
# Problem constants
# ---------------------------------------------------------------------------
B, H, W = 8, 1024, 1024
P, S = 128, 8          # H = P * S
WP = W + 2             # w-padded width (halo cols)
WB = 128               # band width
NB = W // WB
EPS = 1e-6
DT = 1e-2
# input column chunks (padded coords), sized so each DMA descriptor >= 512B
CHUNKS = [(0, 258), (258, 514), (514, 770), (770, 1026)]


def build_nc(ev, ei, kT, kv, ki, ke, Dv, Di, L):
    g = DT * L
    a0 = 1.0 - 4.0 * g * ke     # eta_new = a0*eta + 2g*(fs-u) + g*ke*nsE
    bv = DT * Dv / kT
    bi = DT * Di / kT
    rt2 = float(math.sqrt(2.0))

    nc = bass.Bass()
    # register const APs needed as activation biases (Identity/Ln/Square)
    for cval in (float(ev), float(ei), -1.0):
        if (F32, cval) not in nc.const_aps.aps:
            t = nc.alloc_sbuf_tensor(f"constx-{cval}", [128, 1], F32)
            nc.gpsimd.memset(t.ap(), cval)
            nc.const_aps.aps[(F32, cval)] = t.ap()
    nc.all_engine_barrier()
    dp = nc.declare_dram_parameter
    cvp = dp("cvp", [H, WP], BF16, isOutput=False)
    cip = dp("cip", [H, WP], BF16, isOutput=False)
    etp = dp("etp", [H, WP], BF16, isOutput=False)
    smx = dp("smx", [H, W], BF16, isOutput=False)
    # row-halo arrays: row (8p-1)%1024 ("u") and (8p+8)%1024 ("d"), w-padded
    cvu = dp("cvu", [P, WP], BF16, isOutput=False)
    cvd = dp("cvd", [P, WP], BF16, isOutput=False)
    ciu = dp("ciu", [P, WP], BF16, isOutput=False)
    cid = dp("cid", [P, WP], BF16, isOutput=False)
    etu = dp("etu", [P, WP], BF16, isOutput=False)
    etd = dp("etd", [P, WP], BF16, isOutput=False)
    ocv = dp("cv_new", [H, W], BF16, isOutput=True)
    oci = dp("ci_new", [H, W], BF16, isOutput=True)
    oet = dp("eta_new", [H, W], BF16, isOutput=True)

    cvp3, cip3, etp3 = (x.rearrange("(p s) w -> p s w", s=S) for x in (cvp, cip, etp))
    smx3 = smx.rearrange("(p s) w -> p s w", s=S)
    ocv3, oci3, oet3 = (x.rearrange("(p s) w -> p s w", s=S) for x in (ocv, oci, oet))

    nv, ng, na = nc.vector, nc.gpsimd, nc.scalar

    with TileContext(nc) as tc:
        with tc.tile_pool(name="res", bufs=1) as res:
            # resident input fields and dF fields
            cvm = res.tile([P, S + 2, WP], BF16)
            cim = res.tile([P, S + 2, WP], BF16)
            eta = res.tile([P, S + 2, WP], BF16)
            dFv = res.tile([P, S + 2, WP], BF16)
            dFi = res.tile([P, S + 2, WP], BF16)
            # interleave chunk loads across fields so band-0 deps land first
            for lo, hi in CHUNKS:
                nc.sync.dma_start(out=cvm[:, 1:9, lo:hi], in_=cvp3[:, :, lo:hi])
                nc.sync.dma_start(out=cim[:, 1:9, lo:hi], in_=cip3[:, :, lo:hi])
                nc.sync.dma_start(out=eta[:, 1:9, lo:hi], in_=etp3[:, :, lo:hi])

            # ---------------- pass 1 ----------------
            with tc.tile_pool(name="p1", bufs=1) as p1:
                for t, u, d in ((cvm, cvu, cvd), (cim, ciu, cid), (eta, etu, etd)):
                    nc.sync.dma_start(out=t[:, 0, :], in_=u[:])
                    nc.sync.dma_start(out=t[:, 9, :], in_=d[:])

                with tc.tile_pool(name="p1b", bufs=2) as p1b:
                    def T(tag, dt=BF16):
                        return p1b.tile([P, S, WB], dt, tag=tag, name=tag)

                    smb = oeb = None
                    for b in range(NB):
                        w0 = b * WB          # image col of band start
                        hs = slice(1 + w0, 1 + w0 + WB)   # padded interior cols

                        # S loads and eta_new stores batched 2 bands per DMA
                        # (512B descriptors, full DMA rate)
                        half = (b % 2) * WB
                        if b % 2 == 0:
                            smb = p1b.tile([P, S, 2 * WB], BF16, tag="smb", bufs=1)
                            nc.sync.dma_start(out=smb[:],
                                              in_=smx3[:, :, w0:w0 + 2 * WB])
                            oeb = p1b.tile([P, S, 2 * WB], BF16, tag="oeb")
                        sms = smb[:, :, half:half + WB]

                        def nsum(eng, dst_lr, dst_ud, ft):
                            # dst_lr = left+right ; dst_ud = up+down (band cols)
                            eng.tensor_tensor(dst_lr[:], ft[:, 1:9, w0:w0 + WB],
                                              ft[:, 1:9, w0 + 2:w0 + WB + 2], OP.add)
                            eng.tensor_tensor(dst_ud[:], ft[:, 0:8, hs],
                                              ft[:, 2:10, hs], OP.add)

                        # Act block: logs (fp32 out) then klv/kli (bf16),
                        # then squares (grouped by activation function)
                        ls = p1b.tile([P, S, WB], F32, tag="ls", name="ls", bufs=1)
                        lv = p1b.tile([P, S, WB], F32, tag="lv", name="lv", bufs=1)
                        li = p1b.tile([P, S, WB], F32, tag="li", name="li", bufs=1)
                        na.activation(ls[:], sms, AF.Ln, bias=0.0, scale=1.0)
                        na.activation(lv[:], cvm[:, 1:9, hs], AF.Ln, bias=0.0, scale=1.0)
                        na.activation(li[:], cim[:, 1:9, hs], AF.Ln, bias=0.0, scale=1.0)
                        # klv = kT*lv + Ev ; kli = kT*li + Ei  (bf16 out)
                        klv, kli = T("klv"), T("kli")
                        na.activation(klv[:], lv[:], AF.Identity, bias=float(ev),
                                      scale=kT)
                        na.activation(kli[:], li[:], AF.Identity, bias=float(ei),
                                      scale=kT)
                        h, j2, b2, fv = T("h"), T("j2"), T("b2"), T("fv")
                        na.activation(h[:], eta[:, 1:9, hs], AF.Square, bias=1.0, scale=-1.0)
                        na.activation(j2[:], eta[:, 1:9, hs], AF.Square, bias=0.0, scale=rt2)
                        na.activation(b2[:], cim[:, 1:9, hs], AF.Square, bias=0.0, scale=1.0)
                        na.activation(fv[:], cvm[:, 1:9, hs], AF.Square, bias=1.0, scale=-1.0)
                        # affine prep on Act
                        e0, cm1 = T("e0"), T("cm1")
                        na.activation(e0[:], eta[:, 1:9, hs], AF.Copy, bias=0.0,
                                      scale=a0)
                        na.activation(cm1[:], cvm[:, 1:9, hs], AF.Identity,
                                      bias=-1.0, scale=1.0)

                        # klsp = kT*ls (Act, bf16 out)
                        klsp = T("klsp")
                        na.activation(klsp[:], ls[:], AF.Copy, bias=0.0, scale=kT)
                        # Pv = klv - klsp ; Pi = kli - klsp  (in place over klv/kli)
                        nv.tensor_tensor(klv[:], klv[:], klsp[:], OP.subtract)
                        nv.tensor_tensor(kli[:], kli[:], klsp[:], OP.subtract)
                        pv, pi = klv, kli

                        # neighbor sums (UD of eta on Pool, rest on DVE)
                        lr, ud, nse = T("lr"), T("ud"), T("nse")
                        nsum(nv, lr, ud, eta)
                        nv.tensor_tensor(nse[:], lr[:], ud[:], OP.add)
                        nsv, nsi = T("nsv"), T("nsi")
                        nsum(nv, lr, ud, cvm)
                        nv.tensor_tensor(nsv[:], lr[:], ud[:], OP.add)
                        nsum(nv, lr, ud, cim)
                        nv.tensor_tensor(nsi[:], lr[:], ud[:], OP.add)

                        # fs = cv*Pv + ci*Pi + klsp
                        f1, f2 = T("f1"), T("f2")
                        nv.tensor_tensor(f1[:], cvm[:, 1:9, hs], pv[:], OP.mult)
                        nv.tensor_tensor(f2[:], cim[:, 1:9, hs], pi[:], OP.mult)
                        nv.tensor_tensor(f1[:], f1[:], f2[:], OP.add)
                        fs = T("fs")
                        nv.tensor_tensor(fs[:], f1[:], klsp[:], OP.add)

                        # fv = (cv-1)^2 + ci^2 ; u = eta*(fs+fv) ; w = fs-u
                        nv.tensor_tensor(fv[:], fv[:], b2[:], OP.add)
                        nv.tensor_tensor(fv[:], fs[:], fv[:], OP.add)
                        nv.tensor_tensor(fv[:], eta[:, 1:9, hs], fv[:], OP.mult)
                        nv.tensor_tensor(fv[:], fs[:], fv[:], OP.subtract)
                        w = fv

                        # eta_new = clip(a0*eta + 2g*w + g*ke*nsE)
                        # scale pieces on Pool (tensor_scalar), adds on DVE
                        ng.tensor_scalar(w[:], w[:], 2.0 * g, None, OP.mult)
                        ng.tensor_scalar(nse[:], nse[:], g * ke, None, OP.mult)
                        nv.tensor_tensor(e0[:], e0[:], w[:], OP.add)
                        nv.tensor_tensor(e0[:], e0[:], nse[:], OP.add)
                        ng.tensor_scalar(oeb[:, :, half:half + WB], e0[:],
                                         0.0, 1.0, OP.max, OP.min)
                        if b % 2 == 1:
                            nc.sync.dma_start(out=oet3[:, :, w0 - WB:w0 + WB],
                                              in_=oeb[:])

                        # dFv = h*Pv + j2*(cv-1) - kv*nsv + 4kv*cv
                        mv, nv_ = T("mv"), T("nv_")
                        nv.tensor_tensor(mv[:], h[:], pv[:], OP.mult)
                        nv.tensor_tensor(nv_[:], cm1[:], j2[:], OP.mult)
                        nv.tensor_tensor(mv[:], mv[:], nv_[:], OP.add)
                        ng.tensor_scalar(nsv[:], nsv[:], -kv, None, OP.mult)
                        s4 = p1b.tile([P, S, WB], BF16, tag="s4", name="s4", bufs=1)
                        ng.tensor_scalar(s4[:], cvm[:, 1:9, hs], 4.0 * kv, None,
                                         OP.mult)
                        nv.tensor_tensor(mv[:], mv[:], nsv[:], OP.add)
                        nv.tensor_tensor(dFv[:, 1:9, hs], mv[:], s4[:], OP.add)

                        # dFi = h*Pi + j2*ci - ki*nsi + 4ki*ci
                        mi, ni_ = T("mv"), T("nv_")
                        nv.tensor_tensor(mi[:], h[:], pi[:], OP.mult)
                        nv.tensor_tensor(ni_[:], cim[:, 1:9, hs], j2[:], OP.mult)
                        nv.tensor_tensor(mi[:], mi[:], ni_[:], OP.add)
                        ng.tensor_scalar(nsi[:], nsi[:], -ki, None, OP.mult)
                        s4b = p1b.tile([P, S, WB], BF16, tag="s4", name="s4", bufs=1)
                        ng.tensor_scalar(s4b[:], cim[:, 1:9, hs], 4.0 * ki, None,
                                         OP.mult)
                        nv.tensor_tensor(mi[:], mi[:], nsi[:], OP.add)
                        nv.tensor_tensor(dFi[:, 1:9, hs], mi[:], s4b[:], OP.add)

            # ---------------- dF halo fill ----------------
            with tc.tile_pool(name="p2", bufs=1) as p2:
                for t in (dFv, dFi):
                    nv.tensor_copy(t[:, 1:9, 0:1], t[:, 1:9, W:W + 1])
                    nv.tensor_copy(t[:, 1:9, W + 1:W + 2], t[:, 1:9, 1:2])
                # row-halo DMAs (into rows 0/9 of the dF tiles) read only
                # interior cols and go in column halves, so pass-2 band 0
                # isn't gated on the whole fill.
                for lo, hi in ((1, 514), (514, W + 1)):
                    for src in (dFv, dFi):
                        nc.sync.dma_start(out=src[1:P, 0, lo:hi],
                                          in_=src[0:P - 1, 8, lo:hi])
                        nc.sync.dma_start(out=src[0:1, 0, lo:hi],
                                          in_=src[P - 1:P, 8, lo:hi])
                        nc.sync.dma_start(out=src[0:P - 1, 9, lo:hi],
                                          in_=src[1:P, 1, lo:hi])
                        nc.sync.dma_start(out=src[P - 1:P, 9, lo:hi],
                                          in_=src[0:1, 1, lo:hi])

                # ---------------- pass 2 ----------------
                with tc.tile_pool(name="p2b", bufs=2) as p2b:
                    def T2(tag):
                        return p2b.tile([P, S, WB], BF16, tag=tag, name=tag)

                    obs = {}
                    for b in range(NB):
                        w0 = b * WB
                        hs = slice(1 + w0, 1 + w0 + WB)
                        half = (b % 2) * WB
                        if b % 2 == 0:
                            obs["obv"] = p2b.tile([P, S, 2 * WB], BF16, tag="obv", name="obv")
                            obs["obi"] = p2b.tile([P, S, 2 * WB], BF16, tag="obi", name="obi")
                        tags = {}
                        for fz in ("v", "i"):
                            tags[fz] = (
                                T2("l2" + fz), T2("u2" + fz), T2("n2" + fz),
                                p2b.tile([P, S, WB], F32, tag="t1f" + fz, name="t1f"),
                                p2b.tile([P, S, WB], F32, tag="t2f" + fz, name="t2f"),
                            )

                        def lap2(dF, cres, beta, dst_dram, obtag, fz):
                            lr, ud, ns, t1f, t2f = tags[fz]
                            nv.tensor_tensor(lr[:], dF[:, 1:9, w0:w0 + WB],
                                             dF[:, 1:9, w0 + 2:w0 + WB + 2], OP.add)
                            nv.tensor_tensor(ud[:], dF[:, 0:8, hs],
                                             dF[:, 2:10, hs], OP.add)
                            nv.tensor_tensor(ns[:], lr[:], ud[:], OP.add)
                            # new = clip(c * (1 + beta*(ns - 4*dF)))
                            # t1/t2/m/pm in fp32 for precision (cheap: Act and
                            # Pool ops are dtype-free, only 2 DVE ops go 1x)
                            na.activation(t1f[:], ns[:], AF.Identity, bias=1.0,
                                          scale=beta)
                            ng.tensor_scalar(t2f[:], dF[:, 1:9, hs], -4.0 * beta,
                                             None, OP.mult)
                            nv.tensor_tensor(t1f[:], t1f[:], t2f[:], OP.add)
                            nv.tensor_tensor(t1f[:], cres[:, 1:9, hs], t1f[:], OP.mult)
                            ob = obs[obtag]
                            ng.tensor_scalar(ob[:, :, half:half + WB], t1f[:],
                                             0.0, 1.0, OP.max, OP.min)
                            if b % 2 == 1:
                                nc.sync.dma_start(out=dst_dram[:, :, w0 - WB:w0 + WB],
                                                  in_=ob[:])

                        lap2(dFv, cvm, bv, ocv3, "obv", "v")
                        lap2(dFi, cim, bi, oci3, "obi", "i")
    return nc


_NC_CACHE = {}


def _get_nc(params=None):
    global _NC_CACHE
    if params is None:
        if _NC_CACHE:
            return next(iter(_NC_CACHE.values()))
        params = (0.7729, 0.5245, 0.2182, 0.6689, 0.1679, 0.2640,
                  0.7368, 0.8902, 0.1332)
    key = tuple(round(float(x), 9) for x in params)
    if key not in _NC_CACHE:
        _NC_CACHE[key] = build_nc(*params)
    return _NC_CACHE[key]


def _pad_w(x):
    out = np.empty((x.shape[0], WP), x.dtype)
    out[:, 1:W + 1] = x
    out[:, 0] = x[:, W - 1]
    out[:, W + 1] = x[:, 0]
    return out


_IDX_U = (np.arange(P) * S - 1) % H
_IDX_D = (np.arange(P) * S + S) % H


def kernel(cv, ci, eta, energy_v0, energy_i0, kBT0, kappa_v0, kappa_i0,
           kappa_eta0, diff_v0, diff_i0, L0):
    cv = np.asarray(cv, np.float32)
    ci = np.asarray(ci, np.float32)
    eta = np.asarray(eta, np.float32)
    ab = lambda v: abs(float(np.asarray(v).reshape(-1)[0])) + 0.001
    ev, ei, kT = ab(energy_v0), ab(energy_i0), ab(kBT0)
    kv, ki, ke = ab(kappa_v0), ab(kappa_i0), ab(kappa_eta0)
    Dv, Di, L = ab(diff_v0), ab(diff_i0), ab(L0)

    in_maps = []
    for i in range(B):
        cvm = np.maximum(cv[i], EPS).astype(BF16_NP)
        cim = np.maximum(ci[i], EPS).astype(BF16_NP)
        et16 = eta[i].astype(BF16_NP)
        smx = np.maximum(1.0 - cv[i] - ci[i], EPS).astype(BF16_NP)
        cvp, cip, etp = _pad_w(cvm), _pad_w(cim), _pad_w(et16)
        in_maps.append({
            "cvp": cvp, "cip": cip, "etp": etp, "smx": smx,
            "cvu": np.ascontiguousarray(cvp[_IDX_U]),
            "cvd": np.ascontiguousarray(cvp[_IDX_D]),
            "ciu": np.ascontiguousarray(cip[_IDX_U]),
            "cid": np.ascontiguousarray(cip[_IDX_D]),
            "etu": np.ascontiguousarray(etp[_IDX_U]),
            "etd": np.ascontiguousarray(etp[_IDX_D]),
        })

    nc = _get_nc((ev, ei, kT, kv, ki, ke, Dv, Di, L))
    res = run_bass_kernel_spmd(nc, in_maps, core_ids=list(range(B)))
    cv_new = np.stack([r["cv_new"] for r in res.results]).astype(np.float32)
    ci_new = np.stack([r["ci_new"] for r in res.results]).astype(np.float32)
    eta_new = np.stack([r["eta_new"] for r in res.results]).astype(np.float32)
    return cv_new, ci_new, eta_new


# revision 40
# speedup vs baseline: 1.0974x; 1.0424x over previous
"""Trainium2 Bass kernel for nn_IrradiationSingleTimestep.

Phase-field irradiation single timestep: 3 fields (cv, ci, eta) of shape
[8, 1024, 1024], 5-point periodic Laplacians (two levels), pointwise
thermodynamics with logs, clipped Euler update.

Sharding: batch-parallel, one batch image per NeuronCore (8 cores).

Strategy (bf16 compute, engine-balanced):
- Host precomputes max(cv,eps), max(ci,eps), max(1-cv-ci,eps) in fp32 and
  rounds to bf16.  The 1-cv-ci cancellation is the only absolute-error
  amplifier through ln(); doing it on host keeps ln() errors relative
  (~2^-9) everywhere.  Logs run on the Act engine with fp32 outputs;
  the pass-2 update chain is fp32 for output-error margin.
- DVE perf modes: tensor_scalar bf16 packed = 4x, tensor_tensor bf16
  packed = 2x, fp32 all-SBUF tensor_scalar = 2x, scalar_tensor_tensor =
  no fast mode (and ILLEGAL on GpSimd in the real ISA).  So tensor x
  tensor ops are plain bf16 tensor_tensors on DVE, standalone scalings
  and clips are tensor_scalars on GpSimd, and affine prep (a*x+b) plus
  logs/squares go to the Act engine.
- All 3 input fields and both dF fields are SBUF-resident; inputs stream
  in as 4 column-chunks per field (>=512B descriptors, full DMA rate)
  overlapped with compute; outputs accumulate in 256-col staging tiles
  and store every second band (512B descriptors, no small-DMA penalty).
- Band temps are double-buffered (bufs=2) so consecutive 128-col bands
  pipeline across DVE/Act/GpSimd.
- Scalar parameters are baked as immediates; the program cache is keyed
  on their values (a param change only costs host compile time).

Layout per core: partition p = h // 8 (128 partitions), free dims =
(s = h % 8, w).  h+-1 stencil reads are free-dim shifts except at
s-block edges, which read halo row tiles ([P, WP]): pass-1 halos are
host-marshalled, pass-2 (dF) halos come from SBUF->SBUF DMA.
"""

import json
import math
import numpy as np
import ml_dtypes

import concourse.bass as bass
import concourse.mybir as mybir
from concourse.tile import TileContext
from concourse.bass_utils import run_bass_kernel_spmd

AF = mybir.ActivationFunctionType
OP = mybir.AluOpType
F32 = mybir.dt.float32
BF16 = mybir.dt.bfloat16
BF16_NP = ml_dtypes.bfloat16

# ---------------------------------------------------------------------------
# Workaround: this container's walrus accepts at most ONE sync wait per
# instruction; Tile merges several.  Split extras onto single-wait Drains.
# ---------------------------------------------------------------------------
def _split_waits_json(bj: bytes) -> bytes:
    m = json.loads(bj)
    for f in m["functions"]:
        for blk in f["blocks"]:
            out = []
            for ins in blk["instructions"]:
                si = ins.get("sync_info")
                waits = (si or {}).get("on_wait") or []
                if len(waits) > 1:
                    for k, w in enumerate(waits[:-1]):
                        out.append({
                            "debug": ins.get("debug", 0),
                            "engine": ins["engine"], "ins": [], "outs": [],
                            "is_reset_sema": False,
                            "name": f"{ins['name']}-wsplit{k}",
                            "opcode": "Drain",
                            "sync_info": {"on_update": [], "on_wait": [w]},
                        })
                    si["on_wait"] = [waits[-1]]
                out.append(ins)
            blk["instructions"] = out
    return json.dumps(m).encode()


if not getattr(bass.Bass, "_wait_split_patched", False):
    _orig_to_json_bytes = bass.Bass.to_json_bytes

    def _patched_to_json_bytes(self) -> bytes:
        return _split_waits_json(_orig_to_json_bytes(self))

    bass.Bass.to_json_bytes = _patched_to_json_bytes
    bass.Bass._wait_split_patched = True

# ---------------------------------------------------------------------------
# Problem constants
# ---------------------------------------------------------------------------
B, H, W = 8, 1024, 1024
P, S = 128, 8          # H = P * S
WP = W + 2             # w-padded width (halo cols)
WB = 128               # band width
NB = W // WB
EPS = 1e-6
DT = 1e-2
# input column chunks (padded coords), sized so each DMA descriptor >= 512B
CHUNKS = [(0, 258), (258, 514), (514, 770), (770, 1026)]


def build_nc(ev, ei, kT, kv, ki, ke, Dv, Di, L):
    g = DT * L
    a0 = 1.0 - 4.0 * g * ke     # eta_new = a0*eta + 2g*(fs-u) + g*ke*nsE
    bv = DT * Dv / kT
    bi = DT * Di / kT
    rt2 = float(math.sqrt(2.0))

    nc = bass.Bass()
    # register const APs needed as activation biases (Identity/Ln/Square)
    for cval in (float(ev), float(ei), -1.0):
        if (F32, cval) not in nc.const_aps.aps:
            t = nc.alloc_sbuf_tensor(f"constx-{cval}", [128, 1], F32)
            nc.gpsimd.memset(t.ap(), cval)
            nc.const_aps.aps[(F32, cval)] = t.ap()
    nc.all_engine_barrier()
    dp = nc.declare_dram_parameter
    cvp = dp("cvp", [H, WP], BF16, isOutput=False)
    cip = dp("cip", [H, WP], BF16, isOutput=False)
    etp = dp("etp", [H, WP], BF16, isOutput=False)
    smx = dp("smx", [H, W], BF16, isOutput=False)
    # row-halo arrays: row (8p-1)%1024 ("u") and (8p+8)%1024 ("d"), w-padded
    cvu = dp("cvu", [P, WP], BF16, isOutput=False)
    cvd = dp("cvd", [P, WP], BF16, isOutput=False)
    ciu = dp("ciu", [P, WP], BF16, isOutput=False)
    cid = dp("cid", [P, WP], BF16, isOutput=False)
    etu = dp("etu", [P, WP], BF16, isOutput=False)
    etd = dp("etd", [P, WP], BF16, isOutput=False)
    ocv = dp("cv_new", [H, W], BF16, isOutput=True)
    oci = dp("ci_new", [H, W], BF16, isOutput=True)
    oet = dp("eta_new", [H, W], BF16, isOutput=True)

    cvp3, cip3, etp3 = (x.rearrange("(p s) w -> p s w", s=S) for x in (cvp, cip, etp))
    smx3 = smx.rearrange("(p s) w -> p s w", s=S)
    ocv3, oci3, oet3 = (x.rearrange("(p s) w -> p s w", s=S) for x in (ocv, oci, oet))

    nv, ng, na = nc.vector, nc.gpsimd, nc.scalar

    with TileContext(nc) as tc:
        with tc.tile_pool(name="res", bufs=1) as res:
            # resident input fields and dF fields
            cvm = res.tile([P, S + 2, WP], BF16)
            cim = res.tile([P, S + 2, WP], BF16)
            eta = res.tile([P, S + 2, WP], BF16)
            dFv = res.tile([P, S + 2, WP], BF16)
            dFi = res.tile([P, S + 2, WP], BF16)
            # issue only band-0's field chunks here; smx band 0, halo rows
            # and the remaining chunks are issued inside the band loop so
            # band 0 unblocks as early as possible
            lo, hi = CHUNKS[0]
            nc.sync.dma_start(out=cvm[:, 1:9, lo:hi], in_=cvp3[:, :, lo:hi])
            nc.sync.dma_start(out=cim[:, 1:9, lo:hi], in_=cip3[:, :, lo:hi])
            nc.sync.dma_start(out=eta[:, 1:9, lo:hi], in_=etp3[:, :, lo:hi])

            # ---------------- pass 1 ----------------
            with tc.tile_pool(name="p1", bufs=1) as p1:

                with tc.tile_pool(name="p1b", bufs=2) as p1b:
                    def T(tag, dt=BF16):
                        return p1b.tile([P, S, WB], dt, tag=tag, name=tag)

                    smb = oeb = None
                    for b in range(NB):
                        w0 = b * WB          # image col of band start
                        hs = slice(1 + w0, 1 + w0 + WB)   # padded interior cols

                        # S loads and eta_new stores batched 2 bands per DMA
                        # (512B descriptors, full DMA rate)
                        half = (b % 2) * WB
                        if b % 2 == 0:
                            smb = p1b.tile([P, S, 2 * WB], BF16, tag="smb", bufs=1)
                            nc.sync.dma_start(out=smb[:],
                                              in_=smx3[:, :, w0:w0 + 2 * WB])
                            oeb = p1b.tile([P, S, 2 * WB], BF16, tag="oeb")
                        sms = smb[:, :, half:half + WB]
                        if b == 0:
                            for t, u, d in ((cvm, cvu, cvd), (cim, ciu, cid),
                                            (eta, etu, etd)):
                                nc.sync.dma_start(out=t[:, 0, :], in_=u[:])
                                nc.sync.dma_start(out=t[:, 9, :], in_=d[:])
                            for lo, hi in CHUNKS[1:]:
                                nc.sync.dma_start(out=cvm[:, 1:9, lo:hi],
                                                  in_=cvp3[:, :, lo:hi])
                                nc.sync.dma_start(out=cim[:, 1:9, lo:hi],
                                                  in_=cip3[:, :, lo:hi])
                                nc.sync.dma_start(out=eta[:, 1:9, lo:hi],
                                                  in_=etp3[:, :, lo:hi])

                        def nsum(eng, dst_lr, dst_ud, ft):
                            # dst_lr = left+right ; dst_ud = up+down (band cols)
                            eng.tensor_tensor(dst_lr[:], ft[:, 1:9, w0:w0 + WB],
                                              ft[:, 1:9, w0 + 2:w0 + WB + 2], OP.add)
                            eng.tensor_tensor(dst_ud[:], ft[:, 0:8, hs],
                                              ft[:, 2:10, hs], OP.add)

                        # Act block: logs (fp32 out) then klv/kli (bf16),
                        # then squares (grouped by activation function)
                        ls = p1b.tile([P, S, WB], F32, tag="ls", name="ls", bufs=1)
                        lv = p1b.tile([P, S, WB], F32, tag="lv", name="lv", bufs=1)
                        li = p1b.tile([P, S, WB], F32, tag="li", name="li", bufs=1)
                        na.activation(ls[:], sms, AF.Ln, bias=0.0, scale=1.0)
                        na.activation(lv[:], cvm[:, 1:9, hs], AF.Ln, bias=0.0, scale=1.0)
                        na.activation(li[:], cim[:, 1:9, hs], AF.Ln, bias=0.0, scale=1.0)
                        # klv = kT*lv + Ev ; kli = kT*li + Ei  (bf16 out)
                        klv, kli = T("klv"), T("kli")
                        na.activation(klv[:], lv[:], AF.Identity, bias=float(ev),
                                      scale=kT)
                        na.activation(kli[:], li[:], AF.Identity, bias=float(ei),
                                      scale=kT)
                        h, j2, b2, fv = T("h"), T("j2"), T("b2"), T("fv")
                        na.activation(h[:], eta[:, 1:9, hs], AF.Square, bias=1.0, scale=-1.0)
                        na.activation(j2[:], eta[:, 1:9, hs], AF.Square, bias=0.0, scale=rt2)
                        na.activation(b2[:], cim[:, 1:9, hs], AF.Square, bias=0.0, scale=1.0)
                        na.activation(fv[:], cvm[:, 1:9, hs], AF.Square, bias=1.0, scale=-1.0)
                        # affine prep on Act
                        e0, cm1 = T("e0"), T("cm1")
                        na.activation(e0[:], eta[:, 1:9, hs], AF.Copy, bias=0.0,
                                      scale=a0)
                        na.activation(cm1[:], cvm[:, 1:9, hs], AF.Identity,
                                      bias=-1.0, scale=1.0)

                        # klsp = kT*ls (Act, bf16 out)
                        klsp = T("klsp")
                        na.activation(klsp[:], ls[:], AF.Copy, bias=0.0, scale=kT)
                        # Pv = klv - klsp ; Pi = kli - klsp  (in place over klv/kli)
                        nv.tensor_tensor(klv[:], klv[:], klsp[:], OP.subtract)
                        nv.tensor_tensor(kli[:], kli[:], klsp[:], OP.subtract)
                        pv, pi = klv, kli

                        # neighbor sums (UD of eta on Pool, rest on DVE)
                        lr, ud, nse = T("lr"), T("ud"), T("nse")
                        nsum(nv, lr, ud, eta)
                        nv.tensor_tensor(nse[:], lr[:], ud[:], OP.add)
                        nsv, nsi = T("nsv"), T("nsi")
                        nsum(nv, lr, ud, cvm)
                        nv.tensor_tensor(nsv[:], lr[:], ud[:], OP.add)
                        nsum(nv, lr, ud, cim)
                        nv.tensor_tensor(nsi[:], lr[:], ud[:], OP.add)

                        # fs = cv*Pv + ci*Pi + klsp
                        f1, f2 = T("f1"), T("f2")
                        nv.tensor_tensor(f1[:], cvm[:, 1:9, hs], pv[:], OP.mult)
                        nv.tensor_tensor(f2[:], cim[:, 1:9, hs], pi[:], OP.mult)
                        nv.tensor_tensor(f1[:], f1[:], f2[:], OP.add)
                        fs = T("fs")
                        nv.tensor_tensor(fs[:], f1[:], klsp[:], OP.add)

                        # fv = (cv-1)^2 + ci^2 ; u = eta*(fs+fv) ; w = fs-u
                        nv.tensor_tensor(fv[:], fv[:], b2[:], OP.add)
                        nv.tensor_tensor(fv[:], fs[:], fv[:], OP.add)
                        nv.tensor_tensor(fv[:], eta[:, 1:9, hs], fv[:], OP.mult)
                        nv.tensor_tensor(fv[:], fs[:], fv[:], OP.subtract)
                        w = fv

                        # eta_new = clip(a0*eta + 2g*w + g*ke*nsE)
                        # scale pieces on Pool (tensor_scalar), adds on DVE
                        ng.tensor_scalar(w[:], w[:], 2.0 * g, None, OP.mult)
                        ng.tensor_scalar(nse[:], nse[:], g * ke, None, OP.mult)
                        nv.tensor_tensor(e0[:], e0[:], w[:], OP.add)
                        nv.tensor_tensor(e0[:], e0[:], nse[:], OP.add)
                        ng.tensor_scalar(oeb[:, :, half:half + WB], e0[:],
                                         0.0, 1.0, OP.max, OP.min)
                        if b % 2 == 1:
                            nc.sync.dma_start(out=oet3[:, :, w0 - WB:w0 + WB],
                                              in_=oeb[:])

                        # dFv = h*Pv + j2*(cv-1) - kv*nsv + 4kv*cv
                        mv, nv_ = T("mv"), T("nv_")
                        nv.tensor_tensor(mv[:], h[:], pv[:], OP.mult)
                        nv.tensor_tensor(nv_[:], cm1[:], j2[:], OP.mult)
                        nv.tensor_tensor(mv[:], mv[:], nv_[:], OP.add)
                        ng.tensor_scalar(nsv[:], nsv[:], -kv, None, OP.mult)
                        s4 = p1b.tile([P, S, WB], BF16, tag="s4", name="s4", bufs=1)
                        ng.tensor_scalar(s4[:], cvm[:, 1:9, hs], 4.0 * kv, None,
                                         OP.mult)
                        nv.tensor_tensor(mv[:], mv[:], nsv[:], OP.add)
                        nv.tensor_tensor(dFv[:, 1:9, hs], mv[:], s4[:], OP.add)

                        # dFi = h*Pi + j2*ci - ki*nsi + 4ki*ci
                        mi, ni_ = T("mv"), T("nv_")
                        nv.tensor_tensor(mi[:], h[:], pi[:], OP.mult)
                        nv.tensor_tensor(ni_[:], cim[:, 1:9, hs], j2[:], OP.mult)
                        nv.tensor_tensor(mi[:], mi[:], ni_[:], OP.add)
                        ng.tensor_scalar(nsi[:], nsi[:], -ki, None, OP.mult)
                        s4b = p1b.tile([P, S, WB], BF16, tag="s4", name="s4", bufs=1)
                        ng.tensor_scalar(s4b[:], cim[:, 1:9, hs], 4.0 * ki, None,
                                         OP.mult)
                        nv.tensor_tensor(mi[:], mi[:], nsi[:], OP.add)
                        nv.tensor_tensor(dFi[:, 1:9, hs], mi[:], s4b[:], OP.add)

            # ---------------- dF halo fill ----------------
            with tc.tile_pool(name="p2", bufs=1) as p2:
                for t in (dFv, dFi):
                    nv.tensor_copy(t[:, 1:9, 0:1], t[:, 1:9, W:W + 1])
                    nv.tensor_copy(t[:, 1:9, W + 1:W + 2], t[:, 1:9, 1:2])
                # row-halo DMAs (into rows 0/9 of the dF tiles) read only
                # interior cols and go in column halves, so pass-2 band 0
                # isn't gated on the whole fill.
                for lo, hi in ((1, 514), (514, W + 1)):
                    for src in (dFv, dFi):
                        nc.sync.dma_start(out=src[1:P, 0, lo:hi],
                                          in_=src[0:P - 1, 8, lo:hi])
                        nc.sync.dma_start(out=src[0:1, 0, lo:hi],
                                          in_=src[P - 1:P, 8, lo:hi])
                        nc.sync.dma_start(out=src[0:P - 1, 9, lo:hi],
                                          in_=src[1:P, 1, lo:hi])
                        nc.sync.dma_start(out=src[P - 1:P, 9, lo:hi],
                                          in_=src[0:1, 1, lo:hi])

                # ---------------- pass 2 ----------------
                with tc.tile_pool(name="p2b", bufs=2) as p2b:
                    def T2(tag):
                        return p2b.tile([P, S, WB], BF16, tag=tag, name=tag)

                    obs = {}
                    for b in range(NB):
                        w0 = b * WB
                        hs = slice(1 + w0, 1 + w0 + WB)
                        half = (b % 2) * WB
                        if b % 2 == 0:
                            obs["obv"] = p2b.tile([P, S, 2 * WB], BF16, tag="obv", name="obv")
                            obs["obi"] = p2b.tile([P, S, 2 * WB], BF16, tag="obi", name="obi")
                        tags = {}
                        for fz in ("v", "i"):
                            tags[fz] = (
                                T2("l2" + fz), T2("u2" + fz), T2("n2" + fz),
                                p2b.tile([P, S, WB], F32, tag="t1f" + fz, name="t1f"),
                                p2b.tile([P, S, WB], F32, tag="t2f" + fz, name="t2f"),
                            )

                        def lap2(dF, cres, beta, dst_dram, obtag, fz):
                            lr, ud, ns, t1f, t2f = tags[fz]
                            nv.tensor_tensor(lr[:], dF[:, 1:9, w0:w0 + WB],
                                             dF[:, 1:9, w0 + 2:w0 + WB + 2], OP.add)
                            nv.tensor_tensor(ud[:], dF[:, 0:8, hs],
                                             dF[:, 2:10, hs], OP.add)
                            nv.tensor_tensor(ns[:], lr[:], ud[:], OP.add)
                            # new = clip(c * (1 + beta*(ns - 4*dF)))
                            # t1/t2/m/pm in fp32 for precision (cheap: Act and
                            # Pool ops are dtype-free, only 2 DVE ops go 1x)
                            na.activation(t1f[:], ns[:], AF.Identity, bias=1.0,
                                          scale=beta)
                            ng.tensor_scalar(t2f[:], dF[:, 1:9, hs], -4.0 * beta,
                                             None, OP.mult)
                            nv.tensor_tensor(t1f[:], t1f[:], t2f[:], OP.add)
                            nv.tensor_tensor(t1f[:], cres[:, 1:9, hs], t1f[:], OP.mult)
                            ob = obs[obtag]
                            ng.tensor_scalar(ob[:, :, half:half + WB], t1f[:],
                                             0.0, 1.0, OP.max, OP.min)
                            if b % 2 == 1:
                                nc.sync.dma_start(out=dst_dram[:, :, w0 - WB:w0 + WB],
                                                  in_=ob[:])

                        lap2(dFv, cvm, bv, ocv3, "obv", "v")
                        lap2(dFi, cim, bi, oci3, "obi", "i")
    return nc


_NC_CACHE = {}


def _get_nc(params=None):
    global _NC_CACHE
    if params is None:
        if _NC_CACHE:
            return next(iter(_NC_CACHE.values()))
        params = (0.7729, 0.5245, 0.2182, 0.6689, 0.1679, 0.2640,
                  0.7368, 0.8902, 0.1332)
    key = tuple(round(float(x), 9) for x in params)
    if key not in _NC_CACHE:
        _NC_CACHE[key] = build_nc(*params)
    return _NC_CACHE[key]


def _pad_w(x):
    out = np.empty((x.shape[0], WP), x.dtype)
    out[:, 1:W + 1] = x
    out[:, 0] = x[:, W - 1]
    out[:, W + 1] = x[:, 0]
    return out


_IDX_U = (np.arange(P) * S - 1) % H
_IDX_D = (np.arange(P) * S + S) % H


def kernel(cv, ci, eta, energy_v0, energy_i0, kBT0, kappa_v0, kappa_i0,
           kappa_eta0, diff_v0, diff_i0, L0):
    cv = np.asarray(cv, np.float32)
    ci = np.asarray(ci, np.float32)
    eta = np.asarray(eta, np.float32)
    ab = lambda v: abs(float(np.asarray(v).reshape(-1)[0])) + 0.001
    ev, ei, kT = ab(energy_v0), ab(energy_i0), ab(kBT0)
    kv, ki, ke = ab(kappa_v0), ab(kappa_i0), ab(kappa_eta0)
    Dv, Di, L = ab(diff_v0), ab(diff_i0), ab(L0)

    in_maps = []
    for i in range(B):
        cvm = np.maximum(cv[i], EPS).astype(BF16_NP)
        cim = np.maximum(ci[i], EPS).astype(BF16_NP)
        et16 = eta[i].astype(BF16_NP)
        smx = np.maximum(1.0 - cv[i] - ci[i], EPS).astype(BF16_NP)
        cvp, cip, etp = _pad_w(cvm), _pad_w(cim), _pad_w(et16)
        in_maps.append({
            "cvp": cvp, "cip": cip, "etp": etp, "smx": smx,
            "cvu": np.ascontiguousarray(cvp[_IDX_U]),
            "cvd": np.ascontiguousarray(cvp[_IDX_D]),
            "ciu": np.ascontiguousarray(cip[_IDX_U]),
            "cid": np.ascontiguousarray(cip[_IDX_D]),
            "etu": np.ascontiguousarray(etp[_IDX_U]),
            "etd": np.ascontiguousarray(etp[_IDX_D]),
        })

    nc = _get_nc((ev, ei, kT, kv, ki, ke, Dv, Di, L))
    res = run_bass_kernel_spmd(nc, in_maps, core_ids=list(range(B)))
    cv_new = np.stack([r["cv_new"] for r in res.results]).astype(np.float32)
    ci_new = np.stack([r["ci_new"] for r in res.results]).astype(np.float32)
    eta_new = np.stack([r["eta_new"] for r in res.results]).astype(np.float32)
    return cv_new, ci_new, eta_new


# revision 42
# speedup vs baseline: 1.1095x; 1.0110x over previous
"""Trainium2 Bass kernel for nn_IrradiationSingleTimestep.

Phase-field irradiation single timestep: 3 fields (cv, ci, eta) of shape
[8, 1024, 1024], 5-point periodic Laplacians (two levels), pointwise
thermodynamics with logs, clipped Euler update.

Sharding: batch-parallel, one batch image per NeuronCore (8 cores).

Strategy (bf16 compute, engine-balanced):
- Host precomputes max(cv,eps), max(ci,eps), max(1-cv-ci,eps) in fp32 and
  rounds to bf16.  The 1-cv-ci cancellation is the only absolute-error
  amplifier through ln(); doing it on host keeps ln() errors relative
  (~2^-9) everywhere.  Logs run on the Act engine with fp32 outputs;
  the pass-2 update chain is fp32 for output-error margin.
- DVE perf modes: tensor_scalar bf16 packed = 4x, tensor_tensor bf16
  packed = 2x, fp32 all-SBUF tensor_scalar = 2x, scalar_tensor_tensor =
  no fast mode (and ILLEGAL on GpSimd in the real ISA).  So tensor x
  tensor ops are plain bf16 tensor_tensors on DVE, standalone scalings
  and clips are tensor_scalars on GpSimd, and affine prep (a*x+b) plus
  logs/squares go to the Act engine.
- All 3 input fields and both dF fields are SBUF-resident; inputs stream
  in as 4 column-chunks per field (>=512B descriptors, full DMA rate)
  overlapped with compute; outputs accumulate in 256-col staging tiles
  and store every second band (512B descriptors, no small-DMA penalty).
- Band temps are double-buffered (bufs=2) so consecutive 128-col bands
  pipeline across DVE/Act/GpSimd.
- Scalar parameters are baked as immediates; the program cache is keyed
  on their values (a param change only costs host compile time).

Layout per core: partition p = h // 8 (128 partitions), free dims =
(s = h % 8, w).  h+-1 stencil reads are free-dim shifts except at
s-block edges, which read halo row tiles ([P, WP]): pass-1 halos are
host-marshalled, pass-2 (dF) halos come from SBUF->SBUF DMA.
"""

import json
import math
import numpy as np
import ml_dtypes

import concourse.bass as bass
import concourse.mybir as mybir
from concourse.tile import TileContext
from concourse.bass_utils import run_bass_kernel_spmd

AF = mybir.ActivationFunctionType
OP = mybir.AluOpType
F32 = mybir.dt.float32
BF16 = mybir.dt.bfloat16
BF16_NP = ml_dtypes.bfloat16

# ---------------------------------------------------------------------------
# Workaround: this container's walrus accepts at most ONE sync wait per
# instruction; Tile merges several.  Split extras onto single-wait Drains.
# ---------------------------------------------------------------------------
def _split_waits_json(bj: bytes) -> bytes:
    m = json.loads(bj)
    for f in m["functions"]:
        for blk in f["blocks"]:
            out = []
            for ins in blk["instructions"]:
                si = ins.get("sync_info")
                waits = (si or {}).get("on_wait") or []
                if len(waits) > 1:
                    for k, w in enumerate(waits[:-1]):
                        out.append({
                            "debug": ins.get("debug", 0),
                            "engine": ins["engine"], "ins": [], "outs": [],
                            "is_reset_sema": False,
                            "name": f"{ins['name']}-wsplit{k}",
                            "opcode": "Drain",
                            "sync_info": {"on_update": [], "on_wait": [w]},
                        })
                    si["on_wait"] = [waits[-1]]
                out.append(ins)
            blk["instructions"] = out
    return json.dumps(m).encode()


if not getattr(bass.Bass, "_wait_split_patched", False):
    _orig_to_json_bytes = bass.Bass.to_json_bytes

    def _patched_to_json_bytes(self) -> bytes:
        return _split_waits_json(_orig_to_json_bytes(self))

    bass.Bass.to_json_bytes = _patched_to_json_bytes
    bass.Bass._wait_split_patched = True

# ---------------------------------------------------------------------------
# Problem constants
# ---------------------------------------------------------------------------
B, H, W = 8, 1024, 1024
P, S = 128, 8          # H = P * S
WP = W + 2             # w-padded width (halo cols)
WB = 128               # band width
NB = W // WB
EPS = 1e-6
DT = 1e-2
# input column chunks (padded coords), sized so each DMA descriptor >= 512B
CHUNKS = [(0, 258), (258, 514), (514, 770), (770, 1026)]


def build_nc(ev, ei, kT, kv, ki, ke, Dv, Di, L):
    g = DT * L
    a0 = 1.0 - 4.0 * g * ke     # eta_new = a0*eta + 2g*(fs-u) + g*ke*nsE
    bv = DT * Dv / kT
    bi = DT * Di / kT
    rt2 = float(math.sqrt(2.0))

    nc = bass.Bass()
    # register const APs needed as activation biases (Identity/Ln/Square)
    for cval in (float(ev), float(ei), -1.0):
        if (F32, cval) not in nc.const_aps.aps:
            t = nc.alloc_sbuf_tensor(f"constx-{cval}", [128, 1], F32)
            nc.gpsimd.memset(t.ap(), cval)
            nc.const_aps.aps[(F32, cval)] = t.ap()
    nc.all_engine_barrier()
    dp = nc.declare_dram_parameter
    cvp = dp("cvp", [H, WP], BF16, isOutput=False)
    cip = dp("cip", [H, WP], BF16, isOutput=False)
    etp = dp("etp", [H, WP], BF16, isOutput=False)
    smx = dp("smx", [H, W], BF16, isOutput=False)
    # row-halo arrays: row (8p-1)%1024 ("u") and (8p+8)%1024 ("d"), w-padded
    cvu = dp("cvu", [P, WP], BF16, isOutput=False)
    cvd = dp("cvd", [P, WP], BF16, isOutput=False)
    ciu = dp("ciu", [P, WP], BF16, isOutput=False)
    cid = dp("cid", [P, WP], BF16, isOutput=False)
    etu = dp("etu", [P, WP], BF16, isOutput=False)
    etd = dp("etd", [P, WP], BF16, isOutput=False)
    ocv = dp("cv_new", [H, W], BF16, isOutput=True)
    oci = dp("ci_new", [H, W], BF16, isOutput=True)
    oet = dp("eta_new", [H, W], BF16, isOutput=True)

    cvp3, cip3, etp3 = (x.rearrange("(p s) w -> p s w", s=S) for x in (cvp, cip, etp))
    smx3 = smx.rearrange("(p s) w -> p s w", s=S)
    ocv3, oci3, oet3 = (x.rearrange("(p s) w -> p s w", s=S) for x in (ocv, oci, oet))

    nv, ng, na = nc.vector, nc.gpsimd, nc.scalar

    with TileContext(nc) as tc:
        with tc.tile_pool(name="res", bufs=1) as res:
            # resident input fields and dF fields
            cvm = res.tile([P, S + 2, WP], BF16)
            cim = res.tile([P, S + 2, WP], BF16)
            eta = res.tile([P, S + 2, WP], BF16)
            dFv = res.tile([P, S + 2, WP], BF16)
            dFi = res.tile([P, S + 2, WP], BF16)
            # issue only band-0's field chunks here; smx band 0, halo rows
            # and the remaining chunks are issued inside the band loop so
            # band 0 unblocks as early as possible
            lo, hi = CHUNKS[0]
            nc.sync.dma_start(out=eta[:, 1:9, lo:hi], in_=etp3[:, :, lo:hi])
            nc.sync.dma_start(out=cvm[:, 1:9, lo:hi], in_=cvp3[:, :, lo:hi])
            nc.sync.dma_start(out=cim[:, 1:9, lo:hi], in_=cip3[:, :, lo:hi])

            # ---------------- pass 1 ----------------
            with tc.tile_pool(name="p1", bufs=1) as p1:

                with tc.tile_pool(name="p1b", bufs=2) as p1b:
                    def T(tag, dt=BF16):
                        return p1b.tile([P, S, WB], dt, tag=tag, name=tag)

                    smb = oeb = None
                    for b in range(NB):
                        w0 = b * WB          # image col of band start
                        hs = slice(1 + w0, 1 + w0 + WB)   # padded interior cols

                        # S loads and eta_new stores batched 2 bands per DMA
                        # (512B descriptors, full DMA rate)
                        half = (b % 2) * WB
                        if b % 2 == 0:
                            smb = p1b.tile([P, S, 2 * WB], BF16, tag="smb", bufs=1)
                            nc.sync.dma_start(out=smb[:],
                                              in_=smx3[:, :, w0:w0 + 2 * WB])
                            oeb = p1b.tile([P, S, 2 * WB], BF16, tag="oeb")
                        sms = smb[:, :, half:half + WB]
                        if b == 0:
                            for t, u, d in ((eta, etu, etd), (cvm, cvu, cvd),
                                            (cim, ciu, cid)):
                                nc.sync.dma_start(out=t[:, 0, :], in_=u[:])
                                nc.sync.dma_start(out=t[:, 9, :], in_=d[:])
                            for lo, hi in CHUNKS[1:]:
                                nc.sync.dma_start(out=cvm[:, 1:9, lo:hi],
                                                  in_=cvp3[:, :, lo:hi])
                                nc.sync.dma_start(out=cim[:, 1:9, lo:hi],
                                                  in_=cip3[:, :, lo:hi])
                                nc.sync.dma_start(out=eta[:, 1:9, lo:hi],
                                                  in_=etp3[:, :, lo:hi])

                        def nsum(eng, dst_lr, dst_ud, ft):
                            # dst_lr = left+right ; dst_ud = up+down (band cols)
                            eng.tensor_tensor(dst_lr[:], ft[:, 1:9, w0:w0 + WB],
                                              ft[:, 1:9, w0 + 2:w0 + WB + 2], OP.add)
                            eng.tensor_tensor(dst_ud[:], ft[:, 0:8, hs],
                                              ft[:, 2:10, hs], OP.add)

                        # Act block: logs (fp32 out) then klv/kli (bf16),
                        # then squares (grouped by activation function)
                        ls = p1b.tile([P, S, WB], F32, tag="ls", name="ls", bufs=1)
                        lv = p1b.tile([P, S, WB], F32, tag="lv", name="lv", bufs=1)
                        li = p1b.tile([P, S, WB], F32, tag="li", name="li", bufs=1)
                        na.activation(ls[:], sms, AF.Ln, bias=0.0, scale=1.0)
                        na.activation(lv[:], cvm[:, 1:9, hs], AF.Ln, bias=0.0, scale=1.0)
                        na.activation(li[:], cim[:, 1:9, hs], AF.Ln, bias=0.0, scale=1.0)
                        # klv = kT*lv + Ev ; kli = kT*li + Ei  (bf16 out)
                        klv, kli = T("klv"), T("kli")
                        na.activation(klv[:], lv[:], AF.Identity, bias=float(ev),
                                      scale=kT)
                        na.activation(kli[:], li[:], AF.Identity, bias=float(ei),
                                      scale=kT)
                        h, j2, b2, fv = T("h"), T("j2"), T("b2"), T("fv")
                        na.activation(h[:], eta[:, 1:9, hs], AF.Square, bias=1.0, scale=-1.0)
                        na.activation(j2[:], eta[:, 1:9, hs], AF.Square, bias=0.0, scale=rt2)
                        na.activation(b2[:], cim[:, 1:9, hs], AF.Square, bias=0.0, scale=1.0)
                        na.activation(fv[:], cvm[:, 1:9, hs], AF.Square, bias=1.0, scale=-1.0)
                        # affine prep on Act
                        e0, cm1 = T("e0"), T("cm1")
                        na.activation(e0[:], eta[:, 1:9, hs], AF.Copy, bias=0.0,
                                      scale=a0)
                        na.activation(cm1[:], cvm[:, 1:9, hs], AF.Identity,
                                      bias=-1.0, scale=1.0)

                        # klsp = kT*ls (Act, bf16 out)
                        klsp = T("klsp")
                        na.activation(klsp[:], ls[:], AF.Copy, bias=0.0, scale=kT)
                        # Pv = klv - klsp ; Pi = kli - klsp  (in place over klv/kli)
                        nv.tensor_tensor(klv[:], klv[:], klsp[:], OP.subtract)
                        nv.tensor_tensor(kli[:], kli[:], klsp[:], OP.subtract)
                        pv, pi = klv, kli

                        # neighbor sums (UD of eta on Pool, rest on DVE)
                        lr, ud, nse = T("lr"), T("ud"), T("nse")
                        nsum(nv, lr, ud, eta)
                        nv.tensor_tensor(nse[:], lr[:], ud[:], OP.add)
                        nsv, nsi = T("nsv"), T("nsi")
                        nsum(nv, lr, ud, cvm)
                        nv.tensor_tensor(nsv[:], lr[:], ud[:], OP.add)
                        nsum(nv, lr, ud, cim)
                        nv.tensor_tensor(nsi[:], lr[:], ud[:], OP.add)

                        # fs = cv*Pv + ci*Pi + klsp
                        f1, f2 = T("f1"), T("f2")
                        nv.tensor_tensor(f1[:], cvm[:, 1:9, hs], pv[:], OP.mult)
                        nv.tensor_tensor(f2[:], cim[:, 1:9, hs], pi[:], OP.mult)
                        nv.tensor_tensor(f1[:], f1[:], f2[:], OP.add)
                        fs = T("fs")
                        nv.tensor_tensor(fs[:], f1[:], klsp[:], OP.add)

                        # fv = (cv-1)^2 + ci^2 ; u = eta*(fs+fv) ; w = fs-u
                        nv.tensor_tensor(fv[:], fv[:], b2[:], OP.add)
                        nv.tensor_tensor(fv[:], fs[:], fv[:], OP.add)
                        nv.tensor_tensor(fv[:], eta[:, 1:9, hs], fv[:], OP.mult)
                        nv.tensor_tensor(fv[:], fs[:], fv[:], OP.subtract)
                        w = fv

                        # eta_new = clip(a0*eta + 2g*w + g*ke*nsE)
                        # scale pieces on Pool (tensor_scalar), adds on DVE
                        ng.tensor_scalar(w[:], w[:], 2.0 * g, None, OP.mult)
                        ng.tensor_scalar(nse[:], nse[:], g * ke, None, OP.mult)
                        nv.tensor_tensor(e0[:], e0[:], w[:], OP.add)
                        nv.tensor_tensor(e0[:], e0[:], nse[:], OP.add)
                        ng.tensor_scalar(oeb[:, :, half:half + WB], e0[:],
                                         0.0, 1.0, OP.max, OP.min)
                        if b % 2 == 1:
                            nc.sync.dma_start(out=oet3[:, :, w0 - WB:w0 + WB],
                                              in_=oeb[:])

                        # dFv = h*Pv + j2*(cv-1) - kv*nsv + 4kv*cv
                        mv, nv_ = T("mv"), T("nv_")
                        nv.tensor_tensor(mv[:], h[:], pv[:], OP.mult)
                        nv.tensor_tensor(nv_[:], cm1[:], j2[:], OP.mult)
                        nv.tensor_tensor(mv[:], mv[:], nv_[:], OP.add)
                        ng.tensor_scalar(nsv[:], nsv[:], -kv, None, OP.mult)
                        s4 = p1b.tile([P, S, WB], BF16, tag="s4", name="s4", bufs=1)
                        ng.tensor_scalar(s4[:], cvm[:, 1:9, hs], 4.0 * kv, None,
                                         OP.mult)
                        nv.tensor_tensor(mv[:], mv[:], nsv[:], OP.add)
                        nv.tensor_tensor(dFv[:, 1:9, hs], mv[:], s4[:], OP.add)

                        # dFi = h*Pi + j2*ci - ki*nsi + 4ki*ci
                        mi, ni_ = T("mv"), T("nv_")
                        nv.tensor_tensor(mi[:], h[:], pi[:], OP.mult)
                        nv.tensor_tensor(ni_[:], cim[:, 1:9, hs], j2[:], OP.mult)
                        nv.tensor_tensor(mi[:], mi[:], ni_[:], OP.add)
                        ng.tensor_scalar(nsi[:], nsi[:], -ki, None, OP.mult)
                        s4b = p1b.tile([P, S, WB], BF16, tag="s4", name="s4", bufs=1)
                        ng.tensor_scalar(s4b[:], cim[:, 1:9, hs], 4.0 * ki, None,
                                         OP.mult)
                        nv.tensor_tensor(mi[:], mi[:], nsi[:], OP.add)
                        nv.tensor_tensor(dFi[:, 1:9, hs], mi[:], s4b[:], OP.add)

                        if b in (3, NB - 1):
                            lo2, hi2 = (1, 513) if b == 3 else (513, W + 1)
                            for dsrc in (dFv, dFi):
                                nc.sync.dma_start(out=dsrc[1:P, 0, lo2:hi2],
                                                  in_=dsrc[0:P - 1, 8, lo2:hi2])
                                nc.sync.dma_start(out=dsrc[0:1, 0, lo2:hi2],
                                                  in_=dsrc[P - 1:P, 8, lo2:hi2])
                                nc.sync.dma_start(out=dsrc[0:P - 1, 9, lo2:hi2],
                                                  in_=dsrc[1:P, 1, lo2:hi2])
                                nc.sync.dma_start(out=dsrc[P - 1:P, 9, lo2:hi2],
                                                  in_=dsrc[0:1, 1, lo2:hi2])

            # ---------------- dF halo fill ----------------
            with tc.tile_pool(name="p2", bufs=1) as p2:
                for t in (dFv, dFi):
                    nv.tensor_copy(t[:, 1:9, 0:1], t[:, 1:9, W:W + 1])
                    nv.tensor_copy(t[:, 1:9, W + 1:W + 2], t[:, 1:9, 1:2])

                # ---------------- pass 2 ----------------
                with tc.tile_pool(name="p2b", bufs=2) as p2b:
                    def T2(tag):
                        return p2b.tile([P, S, WB], BF16, tag=tag, name=tag)

                    obs = {}
                    for b in range(NB):
                        w0 = b * WB
                        hs = slice(1 + w0, 1 + w0 + WB)
                        half = (b % 2) * WB
                        if b % 2 == 0:
                            obs["obv"] = p2b.tile([P, S, 2 * WB], BF16, tag="obv", name="obv")
                            obs["obi"] = p2b.tile([P, S, 2 * WB], BF16, tag="obi", name="obi")
                        tags = {}
                        for fz in ("v", "i"):
                            tags[fz] = (
                                T2("l2" + fz), T2("u2" + fz), T2("n2" + fz),
                                p2b.tile([P, S, WB], F32, tag="t1f" + fz, name="t1f"),
                                p2b.tile([P, S, WB], F32, tag="t2f" + fz, name="t2f"),
                            )

                        def lap2(dF, cres, beta, dst_dram, obtag, fz):
                            lr, ud, ns, t1f, t2f = tags[fz]
                            nv.tensor_tensor(lr[:], dF[:, 1:9, w0:w0 + WB],
                                             dF[:, 1:9, w0 + 2:w0 + WB + 2], OP.add)
                            nv.tensor_tensor(ud[:], dF[:, 0:8, hs],
                                             dF[:, 2:10, hs], OP.add)
                            nv.tensor_tensor(ns[:], lr[:], ud[:], OP.add)
                            # new = clip(c * (1 + beta*(ns - 4*dF)))
                            # t1/t2/m/pm in fp32 for precision (cheap: Act and
                            # Pool ops are dtype-free, only 2 DVE ops go 1x)
                            na.activation(t1f[:], ns[:], AF.Identity, bias=1.0,
                                          scale=beta)
                            ng.tensor_scalar(t2f[:], dF[:, 1:9, hs], -4.0 * beta,
                                             None, OP.mult)
                            nv.tensor_tensor(t1f[:], t1f[:], t2f[:], OP.add)
                            nv.tensor_tensor(t1f[:], cres[:, 1:9, hs], t1f[:], OP.mult)
                            ob = obs[obtag]
                            ng.tensor_scalar(ob[:, :, half:half + WB], t1f[:],
                                             0.0, 1.0, OP.max, OP.min)
                            if b % 2 == 1:
                                nc.sync.dma_start(out=dst_dram[:, :, w0 - WB:w0 + WB],
                                                  in_=ob[:])

                        lap2(dFv, cvm, bv, ocv3, "obv", "v")
                        lap2(dFi, cim, bi, oci3, "obi", "i")
    return nc


_NC_CACHE = {}


def _get_nc(params=None):
    global _NC_CACHE
    if params is None:
        if _NC_CACHE:
            return next(iter(_NC_CACHE.values()))
        params = (0.7729, 0.5245, 0.2182, 0.6689, 0.1679, 0.2640,
                  0.7368, 0.8902, 0.1332)
    key = tuple(round(float(x), 9) for x in params)
    if key not in _NC_CACHE:
        _NC_CACHE[key] = build_nc(*params)
    return _NC_CACHE[key]


def _pad_w(x):
    out = np.empty((x.shape[0], WP), x.dtype)
    out[:, 1:W + 1] = x
    out[:, 0] = x[:, W - 1]
    out[:, W + 1] = x[:, 0]
    return out


_IDX_U = (np.arange(P) * S - 1) % H
_IDX_D = (np.arange(P) * S + S) % H


def kernel(cv, ci, eta, energy_v0, energy_i0, kBT0, kappa_v0, kappa_i0,
           kappa_eta0, diff_v0, diff_i0, L0):
    cv = np.asarray(cv, np.float32)
    ci = np.asarray(ci, np.float32)
    eta = np.asarray(eta, np.float32)
    ab = lambda v: abs(float(np.asarray(v).reshape(-1)[0])) + 0.001
    ev, ei, kT = ab(energy_v0), ab(energy_i0), ab(kBT0)
    kv, ki, ke = ab(kappa_v0), ab(kappa_i0), ab(kappa_eta0)
    Dv, Di, L = ab(diff_v0), ab(diff_i0), ab(L0)

    in_maps = []
    for i in range(B):
        cvm = np.maximum(cv[i], EPS).astype(BF16_NP)
        cim = np.maximum(ci[i], EPS).astype(BF16_NP)
        et16 = eta[i].astype(BF16_NP)
        smx = np.maximum(1.0 - cv[i] - ci[i], EPS).astype(BF16_NP)
        cvp, cip, etp = _pad_w(cvm), _pad_w(cim), _pad_w(et16)
        in_maps.append({
            "cvp": cvp, "cip": cip, "etp": etp, "smx": smx,
            "cvu": np.ascontiguousarray(cvp[_IDX_U]),
            "cvd": np.ascontiguousarray(cvp[_IDX_D]),
            "ciu": np.ascontiguousarray(cip[_IDX_U]),
            "cid": np.ascontiguousarray(cip[_IDX_D]),
            "etu": np.ascontiguousarray(etp[_IDX_U]),
            "etd": np.ascontiguousarray(etp[_IDX_D]),
        })

    nc = _get_nc((ev, ei, kT, kv, ki, ke, Dv, Di, L))
    res = run_bass_kernel_spmd(nc, in_maps, core_ids=list(range(B)))
    cv_new = np.stack([r["cv_new"] for r in res.results]).astype(np.float32)
    ci_new = np.stack([r["ci_new"] for r in res.results]).astype(np.float32)
    eta_new = np.stack([r["eta_new"] for r in res.results]).astype(np.float32)
    return cv_new, ci_new, eta_new
